# revision 8
# baseline (speedup 1.0000x reference)
"""DG-block (dual graph-conv) Trainium2 kernel — nn_DG_Block.

Reference per batch item b (B=8, C=128, N=2000, K=9):
  idx1 = top9(knn keys on features_b); idx2 = top9(... motion_b)
  gf_i = graph_feature(features_b, idx_i) -> [2C, N, 9]
  f_i  = conv_bn_relu(1x3 stride 3) -> conv_bn_relu(1x3) on gf_i
  out_b = f1 + delta * f2        [C, N, 1]
BatchNorm pools over the WHOLE batch -> stats are all-reduced across cores.

Sharding: one batch item per NeuronCore (8 cores); conv/BN params replicated;
four [128,2] AllReduces reproduce the exact batch statistics.

Algebra (per branch; w1 [C,2C,1,3] split A_d = w1[:,:C,0,d], B_d = w1[:,C:,0,d]):
  conv1[o,n,t] = (P x_n)[o] - sum_d (B_d x_{idx[n,3t+d]})[o],  P = sum_d A_d+B_d
  (conv biases dropped: BN mean-subtraction cancels them exactly)
  rank-1 neighbor is the point itself -> folded into zA = (P - B_0) x for t=0.
  knn rank key: <x_i, x_j> - |x_j|^2/2  (fp32; monotone per-row transform of
  the reference's key). Diagonal mask -1e30 folded into the negsqh broadcast
  tile: the diag of chunk ci sits at col j with j = c0+p, i.e. j % 128 == p --
  ONE [C,N] tile masks every chunk.

Conv path runs in fp16 (tables, CCE gather-adds, PE transposes, conv2
operands); fp32 PSUM accumulation and fp32 BN statistics. Rel err ~5e-3.

Device pipeline per core:
  tables : yps [cn, 768] = x^T [ -B_0^T | -B_1^T | -B_2^T | zA^T | P^T | P^T ]
           (fp16 matmul); first 384 cols -> DRAM ytab (fp16), last 384 stay
           in SBUF as the conv1 accumulator init [zA|z|z].
  kNN    : pd chunk = X_chunk^T X on PE (fp32) ; DVE fuses PSUM drain +
           (-|x_j|^2/2 - 1e30*diag) add; DVE max8 + max_index.
  conv1  : per chunk THREE multi-offset indirect gathers (one per conv tap
           position d, 2-3 offsets per point) accumulate fp16 ytab rows onto
           the [zA|z|z] tile via SWDGE compute_op=add; per tap-group one fp16
           PE transpose -> stats on ACT (accum_out).
  conv2  : 3 accumulated fp16 matmuls; stats; AllReduce; final Relu-affines,
           f1 + delta*f2 on DVE, DMA out.
"""

import numpy as np

import concourse.bacc as bacc
import concourse.bass as bass
import concourse.mybir as mybir
import concourse.tile as tile
import concourse.bass_utils as bass_utils
from concourse.masks import make_identity

F32 = mybir.dt.float32
F32R = mybir.dt.float32r
F16 = mybir.dt.float16
U32 = mybir.dt.uint32
U16 = mybir.dt.uint16
I16 = mybir.dt.int16
AF = mybir.ActivationFunctionType
ALU = mybir.AluOpType

B = 8
C = 128
N = 2000
EPS = 1e-5
NEG_BIG = -1.0e30

CHUNKS = [(i * 128, min(128, N - i * 128)) for i in range((N + 127) // 128)]
NCH = len(CHUNKS)  # 16
# pd column tiles, 512-aligned so the diagonal block never straddles tiles
JT = [(j * 512, min(512, N - j * 512)) for j in range(4)]
WAVE = 4  # kNN/gather pipelining granularity (chunks)


def build_kernel(delta_nonneg: bool, pd_f32r: bool):
    nc = bacc.Bacc(
        "TRN2",
        target_bir_lowering=False,
        debug=False,
        enable_asserts=False,
        num_devices=B,
    )

    feat_in = nc.dram_tensor("feat", [C, N], F32, kind="ExternalInput").ap()
    mot_in = nc.dram_tensor("mot", [C, N], F32, kind="ExternalInput").ap()
    wb = {}
    for br in (1, 2):
        wb[br] = {
            "nbt": nc.dram_tensor(f"nbt{br}", [C, 768], F16, kind="ExternalInput").ap(),
            "w2t": nc.dram_tensor(f"w2t{br}", [C, 3 * C], F16, kind="ExternalInput").ap(),
            "bn": nc.dram_tensor(f"bn{br}", [C, 4], F32, kind="ExternalInput").ap(),
        }
    delta_in = nc.dram_tensor("delta", [1, 1], F32, kind="ExternalInput").ap()
    out_t = nc.dram_tensor("out", [C, N], F32, kind="ExternalOutput").ap()

    with tile.TileContext(nc) as tc:
        _emit(nc, tc, feat_in, mot_in, wb, delta_in, out_t, delta_nonneg, pd_f32r)
    nc.compile()
    return nc


def _emit(nc, tc, feat_in, mot_in, wb, delta_in, out_t, delta_nonneg, pd_f32r):
    import contextlib

    ctx = contextlib.ExitStack()
    with ctx:
        sb = ctx.enter_context(tc.tile_pool(name="sb", bufs=1))
        pd_ps = ctx.enter_context(tc.tile_pool(name="pd_ps", bufs=2, space="PSUM"))
        st_ps = ctx.enter_context(tc.tile_pool(name="st_ps", bufs=2, space="PSUM"))
        o1_ps = ctx.enter_context(tc.tile_pool(name="o1_ps", bufs=2, space="PSUM"))
        dr = ctx.enter_context(tc.tile_pool(name="dr", bufs=1, space="DRAM"))

        # ---------------- persistent on-chip data ----------------
        x = sb.tile([C, N], F32, name="x")
        nc.sync.dma_start(out=x[:], in_=feat_in)
        m = sb.tile([C, N], F32, name="m")
        nc.sync.dma_start(out=m[:], in_=mot_in)
        xf16 = sb.tile([C, N], F16, name="xf16")
        nc.scalar.activation(out=xf16[:], in_=x[:], func=AF.Copy)

        ident = sb.tile([C, C], F32, name="ident")
        make_identity(nc, ident[:])
        ident16 = sb.tile([C, C], F16, name="ident16")
        nc.scalar.activation(out=ident16[:], in_=ident[:], func=AF.Copy)
        ineg = sb.tile([C, C], F32, name="ineg")
        nc.scalar.activation(out=ineg[:], in_=ident[:], func=AF.Copy, scale=NEG_BIG)
        ones1 = sb.tile([1, C], F32, name="ones1")
        nc.vector.memset(ones1[:], 1.0)
        neghalfc = sb.tile([C, 1], F32, name="neghalfc")
        nc.vector.memset(neghalfc[:], -0.5)

        w = {}
        for br in (1, 2):
            nbt = sb.tile([C, 768], F16, name=f"nbt{br}")
            nc.sync.dma_start(out=nbt[:], in_=wb[br]["nbt"])
            w2t = sb.tile([C, 3 * C], F16, name=f"w2t{br}")
            nc.sync.dma_start(out=w2t[:], in_=wb[br]["w2t"])
            bn = sb.tile([C, 4], F32, name=f"bn{br}")
            nc.sync.dma_start(out=bn[:], in_=wb[br]["bn"])
            w[br] = dict(nbt=nbt, w2t=w2t, bn=bn)

        delta_sb = sb.tile([1, 1], F32, name="delta_sb")
        nc.sync.dma_start(out=delta_sb[:], in_=delta_in)
        dps = st_ps.tile([C, 8], F32, name="dps", tag="stage")
        nc.tensor.matmul(
            out=dps[:, 0:1], lhsT=ones1[:], rhs=delta_sb[0:1, 0:1], start=True, stop=True
        )
        dcol = sb.tile([C, 1], F32, name="dcol")
        nc.scalar.activation(out=dcol[:], in_=dps[:, 0:1], func=AF.Copy)

        ytab = {br: dr.tile([N, 384], F16, name=f"ytab{br}") for br in (1, 2)}
        idx8 = {}
        for s in (1, 2):
            idx8[s] = sb.tile([C, NCH * 8], U16, name=f"idx8_{s}")
            nc.vector.memset(idx8[s][:], 0)
        # dma_gather infra: per conv-tap-position d (d=1 js{1,4,7}, d=2 js{2,5,8},
        # d=0 js{3,6}): NB_d blocks of 128 rows per branch. Gathered rows land at
        # out[g%128, g//128]; the int16 index list is "wrapped in 16 partitions,
        # replicated across cores": idxs16[q, s] = idx_of(g=16s+q).
        D_SPEC = ((1, 3, 0), (2, 3, 1), (0, 2, 2))  # (d, K_d, idx8-col-start)
        g8 = {
            (s, d): sb.tile([C, NCH * k * C], F16, name=f"g8_{s}_{d}")
            for d, k, _ in D_SPEC
            for s in (1, 2)
        }
        idxs16 = {
            (s, d): sb.tile([C, NCH * k * 8], U16, name=f"idxs16_{s}_{d}")
            for d, k, _ in D_SPEC
            for s in (1, 2)
        }

        # ---------------- tables ----------------
        g123 = {1: [], 2: []}

        def tables(br):
            for ci, (c0, cn) in enumerate(CHUNKS):
                yp1 = st_ps.tile([C, 384], F32, name=f"yp1_{br}_{ci}", tag="stage")
                nc.tensor.matmul(
                    out=yp1[:cn, :],
                    lhsT=xf16[:, c0 : c0 + cn],
                    rhs=w[br]["nbt"][:, 0:384],
                    start=True,
                    stop=True,
                )
                ytmp = sb.tile([C, 384], F16, name=f"ytmp_{br}_{ci}", tag="ytmp", bufs=3)
                nc.scalar.activation(out=ytmp[:cn, :], in_=yp1[:cn, :], func=AF.Copy)
                nc.sync.dma_start(out=ytab[br][c0 : c0 + cn, :], in_=ytmp[:cn, :])

                yp2 = st_ps.tile([C, 384], F32, name=f"yp2_{br}_{ci}", tag="stage")
                nc.tensor.matmul(
                    out=yp2[:cn, :],
                    lhsT=xf16[:, c0 : c0 + cn],
                    rhs=w[br]["nbt"][:, 384:768],
                    start=True,
                    stop=True,
                )
                gt = sb.tile([C, 384], F16, name=f"g_{br}_{ci}", tag=f"g{br}", bufs=NCH)
                nc.scalar.activation(out=gt[:cn, :], in_=yp2[:cn, :], func=AF.Copy)
                g123[br].append(gt)

        # ---------------- bulk neighbor gathers (dma_gather) ----------------
        def gather_branch(br, which):
            for d, k, s0 in D_SPEC:
                nb = NCH * k
                # contiguous per-d index blocks (DVE copy from strided idx8 view)
                perm = sb.tile([C, nb], U16, name=f"perm_{which}_{d}", tag="perm", bufs=3)
                nc.vector.tensor_scalar_add(
                    perm[:].rearrange("p (c j) -> p c j", c=NCH),
                    idx8[which][:].rearrange("p (c j) -> p c j", c=NCH)[
                        :, :, s0 : 8 : 3
                    ],
                    0,
                )
                # interleave into the 16-partition-wrapped layout via DRAM:
                # scratchI[q, 8b+r] = perm[16r+q, b]
                scr = dr.tile([16, 8 * nb], U16, name=f"scr_{which}_{d}")
                for r in range(8):
                    ov = scr[:].rearrange("q (b r) -> q b r", r=8)[:, :, r].unsqueeze(2)
                    nc.gpsimd.dma_start(
                        out=ov, in_=perm[16 * r : 16 * (r + 1), :].unsqueeze(2)
                    )
                # replicate to all 8 16-partition groups
                for kk in range(8):
                    nc.gpsimd.dma_start(
                        out=idxs16[(which, d)][16 * kk : 16 * (kk + 1), :], in_=scr[:]
                    )
                # HW ucode caps one instruction at 1024 indices (8 blocks)
                for i0 in range(0, nb, 8):
                    ib = min(8, nb - i0)
                    nc.gpsimd.dma_gather(
                        out_ap=g8[(which, d)][
                            :, i0 * C : (i0 + ib) * C
                        ].rearrange("p (b n) -> p b n", n=C),
                        in_ap=ytab[br][:, d * C : (d + 1) * C],
                        idxs_ap=idxs16[(which, d)][:, i0 * 8 : (i0 + ib) * 8].bitcast(I16),
                        num_idxs=ib * C,
                        num_idxs_reg=ib * C,
                        elem_size=C,
                        elem_step=384,
                    )

        def knn(src, which, br):
            # xsq = src*src (ACT); sqrow = -0.5 * colsum(xsq) (PE)
            xsq = sb.tile([C, N], F32, name=f"xsq_{which}", tag="xsq", bufs=1)
            nc.scalar.activation(out=xsq[:], in_=src[:], func=AF.Square)
            sqrow = sb.tile([1, N], F32, name=f"sqrow_{which}", tag="sqrow", bufs=1)
            for j0, jn in JT:
                sqps = st_ps.tile([1, 512], F32, name=f"sqps_{which}_{j0}", tag="stage")
                nc.tensor.matmul(
                    out=sqps[0:1, :jn],
                    lhsT=neghalfc[:],
                    rhs=xsq[:, j0 : j0 + jn],
                    start=True,
                    stop=True,
                )
                nc.scalar.activation(
                    out=sqrow[0:1, j0 : j0 + jn], in_=sqps[0:1, :jn], func=AF.Copy
                )
            # negsqh broadcast tile (DVE drains PSUM)
            nsd = sb.tile([C, N], F32, name=f"nsd_{which}", tag="nsd", bufs=1)
            for j0, jn in JT:
                nps = st_ps.tile([C, 512], F32, name=f"nps_{which}_{j0}", tag="stage")
                nc.tensor.matmul(
                    out=nps[:, :jn],
                    lhsT=ones1[:],
                    rhs=sqrow[0:1, j0 : j0 + jn],
                    start=True,
                    stop=True,
                )
                nc.scalar.activation(
                    out=nsd[:, j0 : j0 + jn], in_=nps[:, :jn], func=AF.Copy
                )

            halves = [(0, [JT[0], JT[1]]), (1024, [JT[2], JT[3]])]
            if True:
                for ci, (c0, cn) in enumerate(CHUNKS):
                    pdt = sb.tile([C, N], F32, name=f"pd_{which}_{ci}", tag="pd", bufs=2)
                    for h0, jts in halves:
                        pps = pd_ps.tile(
                            [C, 1024], F32, name=f"pps_{which}_{ci}_{h0}", tag="pdps"
                        )
                        off = 0
                        for j0, jn in jts:
                            if pd_f32r:
                                nc.tensor.matmul(
                                    out=pps[:cn, off : off + jn],
                                    lhsT=src[:, c0 : c0 + cn].bitcast(F32R),
                                    rhs=src[:, j0 : j0 + jn].bitcast(F32R),
                                    start=True,
                                    stop=True,
                                )
                            else:
                                nc.tensor.matmul(
                                    out=pps[:cn, off : off + jn],
                                    lhsT=src[:, c0 : c0 + cn],
                                    rhs=src[:, j0 : j0 + jn],
                                    start=True,
                                    stop=True,
                                )
                            off += jn
                        nc.vector.tensor_tensor(
                            out=pdt[:cn, h0 : h0 + off],
                            in0=pps[:cn, 0:off],
                            in1=nsd[:cn, h0 : h0 + off],
                            op=ALU.add,
                        )
                    # mask the current chunk's diagonal block
                    nc.vector.tensor_tensor(
                        out=pdt[:cn, c0 : c0 + cn],
                        in0=pdt[:cn, c0 : c0 + cn],
                        in1=ineg[:cn, :cn],
                        op=ALU.add,
                    )
                    vals8 = sb.tile([C, 8], F32, name=f"v8_{which}_{ci}", tag="v8", bufs=2)
                    nc.vector.max(out=vals8[:cn], in_=pdt[:cn, :])
                    nc.vector.max_index(
                        out=idx8[which][:cn, ci * 8 : ci * 8 + 8],
                        in_max=vals8[:cn],
                        in_values=pdt[:cn, :],
                    )

        # ---------------- conv1 transposes + stats ----------------
        o1_tiles = {}
        stats1 = {}

        def tap_adds(br):
            # fold gathered neighbor rows onto the [zA|z|z] accumulators
            # (fp16 SBUF operands -> DVE 4x mode)
            for ci, (c0, cn) in enumerate(CHUNKS):
                gt = g123[br][ci]
                for d, k, _ in D_SPEC:
                    col0 = 384 - k * C
                    nc.vector.tensor_tensor(
                        out=gt[:cn, col0:384],
                        in0=gt[:cn, col0:384],
                        in1=g8[(br, d)][:cn, ci * k * C : (ci + 1) * k * C],
                        op=ALU.add,
                    )

        def conv1_t(br):
            ol = []
            s1 = sb.tile([C, NCH], F32, name=f"s1c_{br}")
            s2 = sb.tile([C, NCH], F32, name=f"s2c_{br}")
            for ci, (c0, cn) in enumerate(CHUNKS):
                ops = o1_ps.tile([C, 384], F16, name=f"o1ps_{br}_{ci}", tag="o1")
                for t in range(3):
                    nc.tensor.matmul(
                        out=ops[:, t * C : t * C + cn],
                        lhsT=g123[br][ci][:cn, t * C : (t + 1) * C],
                        rhs=ident16[:cn, :cn],
                        is_transpose=True,
                        start=True,
                        stop=True,
                        skip_group_check=True,
                    )
                src_ap = ops[:, 0:384].rearrange("p (t n) -> p t n", t=3)[:, :, :cn]
                ot = sb.tile([C, 384], F16, name=f"o1_{br}_{ci}", tag=f"o1{br}", bufs=NCH)
                dst_ap = ot[:, 0:384].rearrange("p (t n) -> p t n", t=3)[:, :, :cn]
                nc.scalar.activation(
                    out=dst_ap, in_=src_ap, func=AF.Copy, accum_out=s1[:, ci : ci + 1]
                )
                osq = sb.tile([C, 384], F16, name=f"o1sq_{br}_{ci}", tag="o1sq", bufs=2)
                sq_ap = osq[:, 0:384].rearrange("p (t n) -> p t n", t=3)[:, :, :cn]
                nc.scalar.activation(
                    out=sq_ap, in_=src_ap, func=AF.Square, accum_out=s2[:, ci : ci + 1]
                )
                ol.append(ot)
            o1_tiles[br] = ol
            stats1[br] = (s1, s2)

        # ---------------- allreduce + affine computation ----------------
        def allreduce_affine(stats_br, m_count, bn_cols, round_id, br):
            s1, s2 = stats_br
            arq = sb.tile([C, 2], F32, name=f"arq{round_id}")
            nc.vector.reduce_sum(out=arq[:, 0:1], in_=s1[:], axis=mybir.AxisListType.X)
            nc.vector.reduce_sum(out=arq[:, 1:2], in_=s2[:], axis=mybir.AxisListType.X)
            ar_in = dr.tile([C, 2], F32, name=f"arin{round_id}")
            ar_out = dr.tile([C, 2], F32, name=f"arout{round_id}", addr_space="Shared")
            nc.sync.dma_start(out=ar_in[:], in_=arq[:])
            nc.gpsimd.collective_compute(
                "AllReduce",
                ALU.add,
                replica_groups=[list(range(B))],
                ins=[ar_in[:].opt()],
                outs=[ar_out[:].opt()],
            )
            art = sb.tile([C, 2], F32, name=f"art{round_id}")
            nc.sync.dma_start(out=art[:], in_=ar_out[:])

            inv_m = 1.0 / float(m_count)
            gcol = w[br]["bn"][:, bn_cols[0] : bn_cols[0] + 1]
            bcol = w[br]["bn"][:, bn_cols[1] : bn_cols[1] + 1]
            mean = sb.tile([C, 1], F32, name=f"mean{round_id}_{br}")
            nc.vector.tensor_scalar_mul(mean[:], art[:, 0:1], inv_m)
            ey2 = sb.tile([C, 1], F32, name=f"ey2{round_id}_{br}")
            nc.vector.tensor_scalar_mul(ey2[:], art[:, 1:2], inv_m)
            var = sb.tile([C, 1], F32, name=f"var{round_id}_{br}")
            nc.vector.tensor_tensor(out=var[:], in0=mean[:], in1=mean[:], op=ALU.mult)
            nc.vector.tensor_tensor(out=var[:], in0=ey2[:], in1=var[:], op=ALU.subtract)
            nc.vector.tensor_scalar_add(var[:], var[:], EPS)
            rv = sb.tile([C, 1], F32, name=f"rv{round_id}_{br}")
            nc.vector.reciprocal(rv[:], var[:])
            rstd = sb.tile([C, 1], F32, name=f"rstd{round_id}_{br}")
            nc.scalar.activation(out=rstd[:], in_=rv[:], func=AF.Sqrt)
            a_col = sb.tile([C, 1], F32, name=f"acol{round_id}_{br}")
            nc.vector.tensor_tensor(out=a_col[:], in0=gcol, in1=rstd[:], op=ALU.mult)
            c_col = sb.tile([C, 1], F32, name=f"ccol{round_id}_{br}")
            nc.vector.tensor_tensor(out=c_col[:], in0=mean[:], in1=a_col[:], op=ALU.mult)
            nc.vector.tensor_tensor(out=c_col[:], in0=bcol, in1=c_col[:], op=ALU.subtract)
            return (a_col, c_col)

        # ---------------- conv2 + stats ----------------
        o2_tiles = {}
        stats2 = {}

        def conv2(br, aff):
            a_col, c_col = aff
            ol = []
            s1 = sb.tile([C, NCH], F32, name=f"s1d_{br}")
            s2 = sb.tile([C, NCH], F32, name=f"s2d_{br}")
            for ci, (c0, cn) in enumerate(CHUNKS):
                ot = o1_tiles[br][ci]
                o1r_ap = ot[:, 0:384].rearrange("p (t n) -> p t n", t=3)[:, :, :cn]
                nc.scalar.activation(
                    out=o1r_ap, in_=o1r_ap, func=AF.Relu, scale=a_col[:], bias=c_col[:]
                )
                o2ps = st_ps.tile([C, 128], F32, name=f"o2ps_{br}_{ci}", tag="stage")
                for d in range(3):
                    nc.tensor.matmul(
                        out=o2ps[:, :cn],
                        lhsT=w[br]["w2t"][:, d * C : (d + 1) * C],
                        rhs=ot[:, d * C : d * C + cn],
                        start=(d == 0),
                        stop=(d == 2),
                    )
                o2 = sb.tile([C, C], F32, name=f"o2_{br}_{ci}", tag=f"o2{br}", bufs=NCH)
                nc.scalar.activation(
                    out=o2[:, :cn],
                    in_=o2ps[:, :cn],
                    func=AF.Copy,
                    accum_out=s1[:, ci : ci + 1],
                )
                osq = sb.tile([C, C], F32, name=f"o2sq_{br}_{ci}", tag="o2sq", bufs=2)
                nc.scalar.activation(
                    out=osq[:, :cn],
                    in_=o2ps[:, :cn],
                    func=AF.Square,
                    accum_out=s2[:, ci : ci + 1],
                )
                ol.append(o2)
            o2_tiles[br] = ol
            stats2[br] = (s1, s2)

        # ---------------- emit ----------------
        tables(1)
        knn(x, 1, 1)
        gather_branch(1, 1)
        tables(2)
        knn(m, 2, 2)
        gather_branch(2, 2)

        tap_adds(1)
        conv1_t(1)
        tap_adds(2)
        conv1_t(2)

        aff1_1 = allreduce_affine(stats1[1], B * N * 3, (0, 1), "1a", 1)
        conv2(1, aff1_1)
        aff1_2 = allreduce_affine(stats1[2], B * N * 3, (0, 1), "1b", 2)
        conv2(2, aff1_2)

        aff2_1 = allreduce_affine(stats2[1], B * N, (2, 3), "2a", 1)
        a1, c1 = aff2_1
        f1_tiles = []
        for ci, (c0, cn) in enumerate(CHUNKS):
            f1t = o2_tiles[1][ci]
            nc.scalar.activation(
                out=f1t[:, :cn],
                in_=f1t[:, :cn],
                func=AF.Relu,
                scale=a1[:],
                bias=c1[:],
            )
            f1_tiles.append(f1t)

        aff2_2 = allreduce_affine(stats2[2], B * N, (2, 3), "2b", 2)
        a2, c2 = aff2_2
        if delta_nonneg:
            a2d = sb.tile([C, 1], F32, name="a2d")
            nc.vector.tensor_tensor(out=a2d[:], in0=a2[:], in1=dcol[:], op=ALU.mult)
            c2d = sb.tile([C, 1], F32, name="c2d")
            nc.vector.tensor_tensor(out=c2d[:], in0=c2[:], in1=dcol[:], op=ALU.mult)
        for ci, (c0, cn) in enumerate(CHUNKS):
            f1t = f1_tiles[ci]
            f2t = sb.tile([C, C], F32, name=f"f2_{ci}", tag="f2", bufs=2)
            if delta_nonneg:
                nc.scalar.activation(
                    out=f2t[:, :cn],
                    in_=o2_tiles[2][ci][:, :cn],
                    func=AF.Relu,
                    scale=a2d[:],
                    bias=c2d[:],
                )
            else:
                nc.scalar.activation(
                    out=f2t[:, :cn],
                    in_=o2_tiles[2][ci][:, :cn],
                    func=AF.Relu,
                    scale=a2[:],
                    bias=c2[:],
                )
                nc.vector.tensor_scalar_mul(f2t[:, :cn], f2t[:, :cn], dcol[:])
            of = sb.tile([C, C], F32, name=f"of_{ci}", tag="of", bufs=2)
            nc.vector.tensor_tensor(
                out=of[:, :cn], in0=f1t[:, :cn], in1=f2t[:, :cn], op=ALU.add
            )
            nc.sync.dma_start(out=out_t[:, c0 : c0 + cn], in_=of[:, :cn])


# ======================= host side =======================

_CACHE = {}


def _prep_branch(w1, b1, g1, be1, w2, b2, g2, be2):
    w1 = np.asarray(w1, dtype=np.float32)
    w2 = np.asarray(w2, dtype=np.float32)
    A = w1[:, :C, 0, :]  # [o, i, 3]
    Bm = w1[:, C:, 0, :]  # [o, i, 3]
    P = (A + Bm).sum(axis=2)  # [o, i]
    blocks = [np.ascontiguousarray((-Bm[:, :, d]).T) for d in range(3)]  # [i, o]
    zA = np.ascontiguousarray((P - Bm[:, :, 0]).T)
    z = np.ascontiguousarray(P.T)
    nbt_ext = np.concatenate(blocks + [zA, z, z], axis=1).astype(np.float16)  # [C,768]
    w2t = np.ascontiguousarray(
        np.concatenate([w2[:, :, 0, d].T for d in range(3)], axis=1)
    ).astype(np.float16)  # [C, 3C]
    bn = np.ascontiguousarray(
        np.stack(
            [
                np.asarray(g1, np.float32),
                np.asarray(be1, np.float32),
                np.asarray(g2, np.float32),
                np.asarray(be2, np.float32),
            ],
            axis=1,
        )
    )  # [C, 4]
    return nbt_ext, w2t, bn


def kernel(**inputs):
    import os

    features = np.ascontiguousarray(np.asarray(inputs["features"], np.float32))
    motion = np.ascontiguousarray(np.asarray(inputs["motion"], np.float32))
    delta = np.asarray(inputs["delta"], np.float32).reshape(-1)[0]

    nbt1, w2t1, bn1 = _prep_branch(
        inputs["d1_w1"], inputs["d1_b1"], inputs["d1_g1"], inputs["d1_be1"],
        inputs["d1_w2"], inputs["d1_b2"], inputs["d1_g2"], inputs["d1_be2"],
    )
    nbt2, w2t2, bn2 = _prep_branch(
        inputs["d2_w1"], inputs["d2_b1"], inputs["d2_g1"], inputs["d2_be1"],
        inputs["d2_w2"], inputs["d2_b2"], inputs["d2_g2"], inputs["d2_be2"],
    )

    delta_nonneg = bool(delta >= 0.0)
    pd_f32r = bool(int(os.environ.get("DG_PD_F32R", "0")))
    key = ("dg", delta_nonneg, pd_f32r)
    if key not in _CACHE:
        _CACHE[key] = build_kernel(delta_nonneg, pd_f32r)
    nc = _CACHE[key]

    shared = {
        "nbt1": nbt1, "w2t1": w2t1, "bn1": bn1,
        "nbt2": nbt2, "w2t2": w2t2, "bn2": bn2,
        "delta": np.array([[delta]], np.float32),
    }
    in_maps = []
    for c in range(B):
        im = dict(shared)
        im["feat"] = np.ascontiguousarray(features[c, :, :, 0])
        im["mot"] = np.ascontiguousarray(motion[c, :, :, 0])
        in_maps.append(im)

    trace = bool(int(os.environ.get("DG_KERNEL_TRACE", "0")))
    res = bass_utils.run_bass_kernel_spmd(
        nc, in_maps, core_ids=list(range(B)), trace=trace
    )
    global LAST_RESULTS
    LAST_RESULTS = res
    out = np.stack([res.results[c]["out"] for c in range(B)], axis=0)
    return out.reshape(B, C, N, 1).astype(np.float32)


LAST_RESULTS = None


# revision 9
# speedup vs baseline: 1.1017x; 1.1017x over previous
"""DG-block (dual graph-conv) Trainium2 kernel — nn_DG_Block.

Reference per batch item b (B=8, C=128, N=2000, K=9):
  idx1 = top9(knn keys on features_b); idx2 = top9(... motion_b)
  gf_i = graph_feature(features_b, idx_i) -> [2C, N, 9]
  f_i  = conv_bn_relu(1x3 stride 3) -> conv_bn_relu(1x3) on gf_i
  out_b = f1 + delta * f2        [C, N, 1]
BatchNorm pools over the WHOLE batch -> stats are all-reduced across cores.

Sharding: one batch item per NeuronCore (8 cores); conv/BN params replicated;
four [128,2] AllReduces reproduce the exact batch statistics.

Algebra (per branch; w1 [C,2C,1,3] split A_d = w1[:,:C,0,d], B_d = w1[:,C:,0,d]):
  conv1[o,n,t] = (P x_n)[o] - sum_d (B_d x_{idx[n,3t+d]})[o],  P = sum_d A_d+B_d
  (conv biases dropped: BN mean-subtraction cancels them exactly)
  rank-1 neighbor is the point itself -> folded into zA = (P - B_0) x for t=0.
  knn rank key: <x_i, x_j> - |x_j|^2/2  (fp32; monotone per-row transform of
  the reference's key). Diagonal mask -1e30 folded into the negsqh broadcast
  tile: the diag of chunk ci sits at col j with j = c0+p, i.e. j % 128 == p --
  ONE [C,N] tile masks every chunk.

Conv path runs in fp16 (tables, CCE gather-adds, PE transposes, conv2
operands); fp32 PSUM accumulation and fp32 BN statistics. Rel err ~5e-3.

Device pipeline per core:
  tables : yps [cn, 768] = x^T [ -B_0^T | -B_1^T | -B_2^T | zA^T | P^T | P^T ]
           (fp16 matmul); first 384 cols -> DRAM ytab (fp16), last 384 stay
           in SBUF as the conv1 accumulator init [zA|z|z].
  kNN    : pd chunk = X_chunk^T X on PE (fp32) ; DVE fuses PSUM drain +
           (-|x_j|^2/2 - 1e30*diag) add; DVE max8 + max_index.
  conv1  : per chunk THREE multi-offset indirect gathers (one per conv tap
           position d, 2-3 offsets per point) accumulate fp16 ytab rows onto
           the [zA|z|z] tile via SWDGE compute_op=add; per tap-group one fp16
           PE transpose -> stats on ACT (accum_out).
  conv2  : 3 accumulated fp16 matmuls; stats; AllReduce; final Relu-affines,
           f1 + delta*f2 on DVE, DMA out.
"""

import numpy as np

import concourse.bacc as bacc
import concourse.bass as bass
import concourse.mybir as mybir
import concourse.tile as tile
import concourse.bass_utils as bass_utils
from concourse.masks import make_identity

F32 = mybir.dt.float32
F32R = mybir.dt.float32r
F16 = mybir.dt.float16
U32 = mybir.dt.uint32
U16 = mybir.dt.uint16
I16 = mybir.dt.int16
AF = mybir.ActivationFunctionType
ALU = mybir.AluOpType

B = 8
C = 128
N = 2000
EPS = 1e-5
NEG_BIG = -1.0e30

CHUNKS = [(i * 128, min(128, N - i * 128)) for i in range((N + 127) // 128)]
NCH = len(CHUNKS)  # 16
# pd column tiles, 512-aligned so the diagonal block never straddles tiles
JT = [(j * 512, min(512, N - j * 512)) for j in range(4)]
WAVE = 4  # kNN/gather pipelining granularity (chunks)


def build_kernel(delta_nonneg: bool, pd_f32r: bool):
    nc = bacc.Bacc(
        "TRN2",
        target_bir_lowering=False,
        debug=False,
        enable_asserts=False,
        num_devices=B,
    )

    feat_in = nc.dram_tensor("feat", [C, N], F32, kind="ExternalInput").ap()
    mot_in = nc.dram_tensor("mot", [C, N], F32, kind="ExternalInput").ap()
    wb = {}
    for br in (1, 2):
        wb[br] = {
            "nbt": nc.dram_tensor(f"nbt{br}", [C, 768], F16, kind="ExternalInput").ap(),
            "w2t": nc.dram_tensor(f"w2t{br}", [C, 3 * C], F16, kind="ExternalInput").ap(),
            "bn": nc.dram_tensor(f"bn{br}", [C, 4], F32, kind="ExternalInput").ap(),
        }
    delta_in = nc.dram_tensor("delta", [1, 1], F32, kind="ExternalInput").ap()
    out_t = nc.dram_tensor("out", [C, N], F32, kind="ExternalOutput").ap()

    with tile.TileContext(nc) as tc:
        _emit(nc, tc, feat_in, mot_in, wb, delta_in, out_t, delta_nonneg, pd_f32r)
    nc.compile()
    return nc


def _emit(nc, tc, feat_in, mot_in, wb, delta_in, out_t, delta_nonneg, pd_f32r):
    import contextlib

    ctx = contextlib.ExitStack()
    with ctx:
        sb = ctx.enter_context(tc.tile_pool(name="sb", bufs=1))
        pd_ps = ctx.enter_context(tc.tile_pool(name="pd_ps", bufs=2, space="PSUM"))
        st_ps = ctx.enter_context(tc.tile_pool(name="st_ps", bufs=2, space="PSUM"))
        o1_ps = ctx.enter_context(tc.tile_pool(name="o1_ps", bufs=2, space="PSUM"))
        dr = ctx.enter_context(tc.tile_pool(name="dr", bufs=1, space="DRAM"))

        # ---------------- persistent on-chip data ----------------
        x = sb.tile([C, N], F32, name="x")
        nc.sync.dma_start(out=x[:], in_=feat_in)
        m = sb.tile([C, N], F32, name="m")
        nc.sync.dma_start(out=m[:], in_=mot_in)
        xf16 = sb.tile([C, N], F16, name="xf16")
        nc.scalar.activation(out=xf16[:], in_=x[:], func=AF.Copy)

        ident = sb.tile([C, C], F32, name="ident")
        make_identity(nc, ident[:])
        ident16 = sb.tile([C, C], F16, name="ident16")
        nc.scalar.activation(out=ident16[:], in_=ident[:], func=AF.Copy)
        ineg = sb.tile([C, C], F32, name="ineg")
        nc.scalar.activation(out=ineg[:], in_=ident[:], func=AF.Copy, scale=NEG_BIG)
        ones1 = sb.tile([1, C], F32, name="ones1")
        nc.vector.memset(ones1[:], 1.0)
        neghalfc = sb.tile([C, 1], F32, name="neghalfc")
        nc.vector.memset(neghalfc[:], -0.5)

        w = {}
        for br in (1, 2):
            nbt = sb.tile([C, 768], F16, name=f"nbt{br}")
            nc.sync.dma_start(out=nbt[:], in_=wb[br]["nbt"])
            w2t = sb.tile([C, 3 * C], F16, name=f"w2t{br}")
            nc.sync.dma_start(out=w2t[:], in_=wb[br]["w2t"])
            bn = sb.tile([C, 4], F32, name=f"bn{br}")
            nc.sync.dma_start(out=bn[:], in_=wb[br]["bn"])
            w[br] = dict(nbt=nbt, w2t=w2t, bn=bn)

        delta_sb = sb.tile([1, 1], F32, name="delta_sb")
        nc.sync.dma_start(out=delta_sb[:], in_=delta_in)
        dps = st_ps.tile([C, 8], F32, name="dps", tag="stage")
        nc.tensor.matmul(
            out=dps[:, 0:1], lhsT=ones1[:], rhs=delta_sb[0:1, 0:1], start=True, stop=True
        )
        dcol = sb.tile([C, 1], F32, name="dcol")
        nc.scalar.activation(out=dcol[:], in_=dps[:, 0:1], func=AF.Copy)

        ytab = {br: dr.tile([N, 384], F16, name=f"ytab{br}") for br in (1, 2)}
        idx8 = {}
        for s in (1, 2):
            idx8[s] = sb.tile([C, NCH * 8], U16, name=f"idx8_{s}")
            nc.vector.memset(idx8[s][:], 0)
        # dma_gather infra: per conv-tap-position d (d=1 js{1,4,7}, d=2 js{2,5,8},
        # d=0 js{3,6}): NB_d blocks of 128 rows per branch. Gathered rows land at
        # out[g%128, g//128]; the int16 index list is "wrapped in 16 partitions,
        # replicated across cores": idxs16[q, s] = idx_of(g=16s+q).
        D_SPEC = ((1, 3, 0), (2, 3, 1), (0, 2, 2))  # (d, K_d, idx8-col-start)
        g8 = {
            (s, d): sb.tile([C, NCH * k * C], F16, name=f"g8_{s}_{d}")
            for d, k, _ in D_SPEC
            for s in (1, 2)
        }
        idxs16 = {
            (s, d): sb.tile([C, NCH * k * 8], U16, name=f"idxs16_{s}_{d}")
            for d, k, _ in D_SPEC
            for s in (1, 2)
        }

        # ---------------- tables ----------------
        g123 = {1: [], 2: []}

        def tables(br):
            for ci, (c0, cn) in enumerate(CHUNKS):
                yp1 = st_ps.tile([C, 384], F32, name=f"yp1_{br}_{ci}", tag="stage")
                nc.tensor.matmul(
                    out=yp1[:cn, :],
                    lhsT=xf16[:, c0 : c0 + cn],
                    rhs=w[br]["nbt"][:, 0:384],
                    start=True,
                    stop=True,
                )
                ytmp = sb.tile([C, 384], F16, name=f"ytmp_{br}_{ci}", tag="ytmp", bufs=3)
                nc.scalar.activation(out=ytmp[:cn, :], in_=yp1[:cn, :], func=AF.Copy)
                nc.sync.dma_start(out=ytab[br][c0 : c0 + cn, :], in_=ytmp[:cn, :])

                yp2 = st_ps.tile([C, 384], F32, name=f"yp2_{br}_{ci}", tag="stage")
                nc.tensor.matmul(
                    out=yp2[:cn, :],
                    lhsT=xf16[:, c0 : c0 + cn],
                    rhs=w[br]["nbt"][:, 384:768],
                    start=True,
                    stop=True,
                )
                gt = sb.tile([C, 384], F16, name=f"g_{br}_{ci}", tag=f"g{br}", bufs=NCH)
                nc.scalar.activation(out=gt[:cn, :], in_=yp2[:cn, :], func=AF.Copy)
                g123[br].append(gt)

        # ---------------- bulk neighbor gathers (dma_gather) ----------------
        def gather_half(br, which, half):
            """Gather neighbor rows for chunks [half*8, half*8+8) of a branch.

            Per conv-tap-position d: nb blocks of 128 rows; row g of the list
            lands at out[g%128, g//128]. idx list is int16, wrapped in 16
            partitions (idxs16[q, s] = idx_of(g=16s+q)), replicated across the
            8 Q7 cores. The wrap needs cross-partition motion -> DRAM bounce
            (interleave writes + replication readbacks) on the sync queue.
            """
            ch0 = half * (NCH // 2)
            chn = NCH // 2
            for d, k, s0 in D_SPEC:
                nb = chn * k
                perm = sb.tile(
                    [C, nb], U16, name=f"perm_{which}_{half}_{d}", tag="perm", bufs=3
                )
                nc.vector.tensor_scalar_add(
                    perm[:].rearrange("p (c j) -> p c j", c=chn),
                    idx8[which][:].rearrange("p (c j) -> p c j", c=NCH)[
                        :, ch0 : ch0 + chn, s0 : 8 : 3
                    ],
                    0,
                )
                scr = dr.tile([16, 8 * nb], U16, name=f"scr_{which}_{half}_{d}")
                for r in range(8):
                    ov = scr[:].rearrange("q (b r) -> q b r", r=8)[:, :, r].unsqueeze(2)
                    nc.sync.dma_start(
                        out=ov, in_=perm[16 * r : 16 * (r + 1), :].unsqueeze(2)
                    )
                ix = idxs16[(which, d)]
                col0 = ch0 * k * 8
                for kk in range(8):
                    nc.sync.dma_start(
                        out=ix[16 * kk : 16 * (kk + 1), col0 : col0 + nb * 8],
                        in_=scr[:],
                    )
                # HW ucode caps one instruction at 1024 indices (8 blocks)
                b0 = ch0 * k
                for i0 in range(0, nb, 8):
                    ib = min(8, nb - i0)
                    nc.gpsimd.dma_gather(
                        out_ap=g8[(which, d)][
                            :, (b0 + i0) * C : (b0 + i0 + ib) * C
                        ].rearrange("p (b n) -> p b n", n=C),
                        in_ap=ytab[br][:, d * C : (d + 1) * C],
                        idxs_ap=ix[:, (b0 + i0) * 8 : (b0 + i0 + ib) * 8].bitcast(I16),
                        num_idxs=ib * C,
                        num_idxs_reg=ib * C,
                        elem_size=C,
                        elem_step=384,
                    )

        def knn(src, which, br):
            # xsq = src*src (ACT); sqrow = -0.5 * colsum(xsq) (PE)
            xsq = sb.tile([C, N], F32, name=f"xsq_{which}", tag="xsq", bufs=1)
            nc.scalar.activation(out=xsq[:], in_=src[:], func=AF.Square)
            sqrow = sb.tile([1, N], F32, name=f"sqrow_{which}", tag="sqrow", bufs=1)
            for j0, jn in JT:
                sqps = st_ps.tile([1, 512], F32, name=f"sqps_{which}_{j0}", tag="stage")
                nc.tensor.matmul(
                    out=sqps[0:1, :jn],
                    lhsT=neghalfc[:],
                    rhs=xsq[:, j0 : j0 + jn],
                    start=True,
                    stop=True,
                )
                nc.scalar.activation(
                    out=sqrow[0:1, j0 : j0 + jn], in_=sqps[0:1, :jn], func=AF.Copy
                )
            # negsqh broadcast tile (DVE drains PSUM)
            nsd = sb.tile([C, N], F32, name=f"nsd_{which}", tag="nsd", bufs=1)
            for j0, jn in JT:
                nps = st_ps.tile([C, 512], F32, name=f"nps_{which}_{j0}", tag="stage")
                nc.tensor.matmul(
                    out=nps[:, :jn],
                    lhsT=ones1[:],
                    rhs=sqrow[0:1, j0 : j0 + jn],
                    start=True,
                    stop=True,
                )
                nc.scalar.activation(
                    out=nsd[:, j0 : j0 + jn], in_=nps[:, :jn], func=AF.Copy
                )

            halves = [(0, [JT[0], JT[1]]), (1024, [JT[2], JT[3]])]
            if True:
                for ci, (c0, cn) in enumerate(CHUNKS):
                    if ci == NCH // 2:
                        gather_half(br, which, 0)
                    pdt = sb.tile([C, N], F32, name=f"pd_{which}_{ci}", tag="pd", bufs=2)
                    for h0, jts in halves:
                        pps = pd_ps.tile(
                            [C, 1024], F32, name=f"pps_{which}_{ci}_{h0}", tag="pdps"
                        )
                        off = 0
                        for j0, jn in jts:
                            if pd_f32r:
                                nc.tensor.matmul(
                                    out=pps[:cn, off : off + jn],
                                    lhsT=src[:, c0 : c0 + cn].bitcast(F32R),
                                    rhs=src[:, j0 : j0 + jn].bitcast(F32R),
                                    start=True,
                                    stop=True,
                                )
                            else:
                                nc.tensor.matmul(
                                    out=pps[:cn, off : off + jn],
                                    lhsT=src[:, c0 : c0 + cn],
                                    rhs=src[:, j0 : j0 + jn],
                                    start=True,
                                    stop=True,
                                )
                            off += jn
                        nc.vector.tensor_tensor(
                            out=pdt[:cn, h0 : h0 + off],
                            in0=pps[:cn, 0:off],
                            in1=nsd[:cn, h0 : h0 + off],
                            op=ALU.add,
                        )
                    # mask the current chunk's diagonal block
                    nc.vector.tensor_tensor(
                        out=pdt[:cn, c0 : c0 + cn],
                        in0=pdt[:cn, c0 : c0 + cn],
                        in1=ineg[:cn, :cn],
                        op=ALU.add,
                    )
                    vals8 = sb.tile([C, 8], F32, name=f"v8_{which}_{ci}", tag="v8", bufs=2)
                    nc.vector.max(out=vals8[:cn], in_=pdt[:cn, :])
                    nc.vector.max_index(
                        out=idx8[which][:cn, ci * 8 : ci * 8 + 8],
                        in_max=vals8[:cn],
                        in_values=pdt[:cn, :],
                    )
                gather_half(br, which, 1)

        # ---------------- conv1 transposes + stats ----------------
        o1_tiles = {}
        stats1 = {}

        def tap_adds(br):
            # fold gathered neighbor rows onto the [zA|z|z] accumulators
            # (fp16 SBUF operands -> DVE 4x mode)
            for ci, (c0, cn) in enumerate(CHUNKS):
                gt = g123[br][ci]
                for d, k, _ in D_SPEC:
                    col0 = 384 - k * C
                    nc.vector.tensor_tensor(
                        out=gt[:cn, col0:384],
                        in0=gt[:cn, col0:384],
                        in1=g8[(br, d)][:cn, ci * k * C : (ci + 1) * k * C],
                        op=ALU.add,
                    )

        def conv1_t(br):
            ol = []
            s1 = sb.tile([C, NCH], F32, name=f"s1c_{br}")
            s2 = sb.tile([C, NCH], F32, name=f"s2c_{br}")
            for ci, (c0, cn) in enumerate(CHUNKS):
                ops = o1_ps.tile([C, 384], F16, name=f"o1ps_{br}_{ci}", tag="o1")
                for t in range(3):
                    nc.tensor.matmul(
                        out=ops[:, t * C : t * C + cn],
                        lhsT=g123[br][ci][:cn, t * C : (t + 1) * C],
                        rhs=ident16[:cn, :cn],
                        is_transpose=True,
                        start=True,
                        stop=True,
                        skip_group_check=True,
                    )
                src_ap = ops[:, 0:384].rearrange("p (t n) -> p t n", t=3)[:, :, :cn]
                ot = sb.tile([C, 384], F16, name=f"o1_{br}_{ci}", tag=f"o1{br}", bufs=NCH)
                dst_ap = ot[:, 0:384].rearrange("p (t n) -> p t n", t=3)[:, :, :cn]
                nc.scalar.activation(
                    out=dst_ap, in_=src_ap, func=AF.Copy, accum_out=s1[:, ci : ci + 1]
                )
                osq = sb.tile([C, 384], F16, name=f"o1sq_{br}_{ci}", tag="o1sq", bufs=2)
                sq_ap = osq[:, 0:384].rearrange("p (t n) -> p t n", t=3)[:, :, :cn]
                nc.scalar.activation(
                    out=sq_ap, in_=src_ap, func=AF.Square, accum_out=s2[:, ci : ci + 1]
                )
                ol.append(ot)
            o1_tiles[br] = ol
            stats1[br] = (s1, s2)

        # ---------------- allreduce + affine computation ----------------
        def allreduce_affine(stats_br, m_count, bn_cols, round_id, br):
            s1, s2 = stats_br
            arq = sb.tile([C, 2], F32, name=f"arq{round_id}")
            nc.vector.reduce_sum(out=arq[:, 0:1], in_=s1[:], axis=mybir.AxisListType.X)
            nc.vector.reduce_sum(out=arq[:, 1:2], in_=s2[:], axis=mybir.AxisListType.X)
            ar_in = dr.tile([C, 2], F32, name=f"arin{round_id}")
            ar_out = dr.tile([C, 2], F32, name=f"arout{round_id}", addr_space="Shared")
            nc.sync.dma_start(out=ar_in[:], in_=arq[:])
            nc.gpsimd.collective_compute(
                "AllReduce",
                ALU.add,
                replica_groups=[list(range(B))],
                ins=[ar_in[:].opt()],
                outs=[ar_out[:].opt()],
            )
            art = sb.tile([C, 2], F32, name=f"art{round_id}")
            nc.sync.dma_start(out=art[:], in_=ar_out[:])

            inv_m = 1.0 / float(m_count)
            gcol = w[br]["bn"][:, bn_cols[0] : bn_cols[0] + 1]
            bcol = w[br]["bn"][:, bn_cols[1] : bn_cols[1] + 1]
            mean = sb.tile([C, 1], F32, name=f"mean{round_id}_{br}")
            nc.vector.tensor_scalar_mul(mean[:], art[:, 0:1], inv_m)
            ey2 = sb.tile([C, 1], F32, name=f"ey2{round_id}_{br}")
            nc.vector.tensor_scalar_mul(ey2[:], art[:, 1:2], inv_m)
            var = sb.tile([C, 1], F32, name=f"var{round_id}_{br}")
            nc.vector.tensor_tensor(out=var[:], in0=mean[:], in1=mean[:], op=ALU.mult)
            nc.vector.tensor_tensor(out=var[:], in0=ey2[:], in1=var[:], op=ALU.subtract)
            nc.vector.tensor_scalar_add(var[:], var[:], EPS)
            rv = sb.tile([C, 1], F32, name=f"rv{round_id}_{br}")
            nc.vector.reciprocal(rv[:], var[:])
            rstd = sb.tile([C, 1], F32, name=f"rstd{round_id}_{br}")
            nc.scalar.activation(out=rstd[:], in_=rv[:], func=AF.Sqrt)
            a_col = sb.tile([C, 1], F32, name=f"acol{round_id}_{br}")
            nc.vector.tensor_tensor(out=a_col[:], in0=gcol, in1=rstd[:], op=ALU.mult)
            c_col = sb.tile([C, 1], F32, name=f"ccol{round_id}_{br}")
            nc.vector.tensor_tensor(out=c_col[:], in0=mean[:], in1=a_col[:], op=ALU.mult)
            nc.vector.tensor_tensor(out=c_col[:], in0=bcol, in1=c_col[:], op=ALU.subtract)
            return (a_col, c_col)

        # ---------------- conv2 + stats ----------------
        o2_tiles = {}
        stats2 = {}

        def conv2(br, aff):
            a_col, c_col = aff
            ol = []
            s1 = sb.tile([C, NCH], F32, name=f"s1d_{br}")
            s2 = sb.tile([C, NCH], F32, name=f"s2d_{br}")
            for ci, (c0, cn) in enumerate(CHUNKS):
                ot = o1_tiles[br][ci]
                o1r_ap = ot[:, 0:384].rearrange("p (t n) -> p t n", t=3)[:, :, :cn]
                nc.scalar.activation(
                    out=o1r_ap, in_=o1r_ap, func=AF.Relu, scale=a_col[:], bias=c_col[:]
                )
                o2ps = st_ps.tile([C, 128], F32, name=f"o2ps_{br}_{ci}", tag="stage")
                for d in range(3):
                    nc.tensor.matmul(
                        out=o2ps[:, :cn],
                        lhsT=w[br]["w2t"][:, d * C : (d + 1) * C],
                        rhs=ot[:, d * C : d * C + cn],
                        start=(d == 0),
                        stop=(d == 2),
                    )
                o2 = sb.tile([C, C], F32, name=f"o2_{br}_{ci}", tag=f"o2{br}", bufs=NCH)
                nc.scalar.activation(
                    out=o2[:, :cn],
                    in_=o2ps[:, :cn],
                    func=AF.Copy,
                    accum_out=s1[:, ci : ci + 1],
                )
                osq = sb.tile([C, C], F32, name=f"o2sq_{br}_{ci}", tag="o2sq", bufs=2)
                nc.scalar.activation(
                    out=osq[:, :cn],
                    in_=o2ps[:, :cn],
                    func=AF.Square,
                    accum_out=s2[:, ci : ci + 1],
                )
                ol.append(o2)
            o2_tiles[br] = ol
            stats2[br] = (s1, s2)

        # ---------------- emit ----------------
        tables(1)
        knn(x, 1, 1)
        tables(2)
        knn(m, 2, 2)

        tap_adds(1)
        conv1_t(1)
        tap_adds(2)
        conv1_t(2)

        aff1_1 = allreduce_affine(stats1[1], B * N * 3, (0, 1), "1a", 1)
        conv2(1, aff1_1)
        aff1_2 = allreduce_affine(stats1[2], B * N * 3, (0, 1), "1b", 2)
        conv2(2, aff1_2)

        aff2_1 = allreduce_affine(stats2[1], B * N, (2, 3), "2a", 1)
        a1, c1 = aff2_1
        f1_tiles = []
        for ci, (c0, cn) in enumerate(CHUNKS):
            f1t = o2_tiles[1][ci]
            nc.scalar.activation(
                out=f1t[:, :cn],
                in_=f1t[:, :cn],
                func=AF.Relu,
                scale=a1[:],
                bias=c1[:],
            )
            f1_tiles.append(f1t)

        aff2_2 = allreduce_affine(stats2[2], B * N, (2, 3), "2b", 2)
        a2, c2 = aff2_2
        if delta_nonneg:
            a2d = sb.tile([C, 1], F32, name="a2d")
            nc.vector.tensor_tensor(out=a2d[:], in0=a2[:], in1=dcol[:], op=ALU.mult)
            c2d = sb.tile([C, 1], F32, name="c2d")
            nc.vector.tensor_tensor(out=c2d[:], in0=c2[:], in1=dcol[:], op=ALU.mult)
        for ci, (c0, cn) in enumerate(CHUNKS):
            f1t = f1_tiles[ci]
            f2t = sb.tile([C, C], F32, name=f"f2_{ci}", tag="f2", bufs=2)
            if delta_nonneg:
                nc.scalar.activation(
                    out=f2t[:, :cn],
                    in_=o2_tiles[2][ci][:, :cn],
                    func=AF.Relu,
                    scale=a2d[:],
                    bias=c2d[:],
                )
            else:
                nc.scalar.activation(
                    out=f2t[:, :cn],
                    in_=o2_tiles[2][ci][:, :cn],
                    func=AF.Relu,
                    scale=a2[:],
                    bias=c2[:],
                )
                nc.vector.tensor_scalar_mul(f2t[:, :cn], f2t[:, :cn], dcol[:])
            of = sb.tile([C, C], F32, name=f"of_{ci}", tag="of", bufs=2)
            nc.vector.tensor_tensor(
                out=of[:, :cn], in0=f1t[:, :cn], in1=f2t[:, :cn], op=ALU.add
            )
            nc.sync.dma_start(out=out_t[:, c0 : c0 + cn], in_=of[:, :cn])


# ======================= host side =======================

_CACHE = {}


def _prep_branch(w1, b1, g1, be1, w2, b2, g2, be2):
    w1 = np.asarray(w1, dtype=np.float32)
    w2 = np.asarray(w2, dtype=np.float32)
    A = w1[:, :C, 0, :]  # [o, i, 3]
    Bm = w1[:, C:, 0, :]  # [o, i, 3]
    P = (A + Bm).sum(axis=2)  # [o, i]
    blocks = [np.ascontiguousarray((-Bm[:, :, d]).T) for d in range(3)]  # [i, o]
    zA = np.ascontiguousarray((P - Bm[:, :, 0]).T)
    z = np.ascontiguousarray(P.T)
    nbt_ext = np.concatenate(blocks + [zA, z, z], axis=1).astype(np.float16)  # [C,768]
    w2t = np.ascontiguousarray(
        np.concatenate([w2[:, :, 0, d].T for d in range(3)], axis=1)
    ).astype(np.float16)  # [C, 3C]
    bn = np.ascontiguousarray(
        np.stack(
            [
                np.asarray(g1, np.float32),
                np.asarray(be1, np.float32),
                np.asarray(g2, np.float32),
                np.asarray(be2, np.float32),
            ],
            axis=1,
        )
    )  # [C, 4]
    return nbt_ext, w2t, bn


def kernel(**inputs):
    import os

    features = np.ascontiguousarray(np.asarray(inputs["features"], np.float32))
    motion = np.ascontiguousarray(np.asarray(inputs["motion"], np.float32))
    delta = np.asarray(inputs["delta"], np.float32).reshape(-1)[0]

    nbt1, w2t1, bn1 = _prep_branch(
        inputs["d1_w1"], inputs["d1_b1"], inputs["d1_g1"], inputs["d1_be1"],
        inputs["d1_w2"], inputs["d1_b2"], inputs["d1_g2"], inputs["d1_be2"],
    )
    nbt2, w2t2, bn2 = _prep_branch(
        inputs["d2_w1"], inputs["d2_b1"], inputs["d2_g1"], inputs["d2_be1"],
        inputs["d2_w2"], inputs["d2_b2"], inputs["d2_g2"], inputs["d2_be2"],
    )

    delta_nonneg = bool(delta >= 0.0)
    pd_f32r = bool(int(os.environ.get("DG_PD_F32R", "0")))
    key = ("dg", delta_nonneg, pd_f32r)
    if key not in _CACHE:
        _CACHE[key] = build_kernel(delta_nonneg, pd_f32r)
    nc = _CACHE[key]

    shared = {
        "nbt1": nbt1, "w2t1": w2t1, "bn1": bn1,
        "nbt2": nbt2, "w2t2": w2t2, "bn2": bn2,
        "delta": np.array([[delta]], np.float32),
    }
    in_maps = []
    for c in range(B):
        im = dict(shared)
        im["feat"] = np.ascontiguousarray(features[c, :, :, 0])
        im["mot"] = np.ascontiguousarray(motion[c, :, :, 0])
        in_maps.append(im)

    trace = bool(int(os.environ.get("DG_KERNEL_TRACE", "0")))
    res = bass_utils.run_bass_kernel_spmd(
        nc, in_maps, core_ids=list(range(B)), trace=trace
    )
    global LAST_RESULTS
    LAST_RESULTS = res
    out = np.stack([res.results[c]["out"] for c in range(B)], axis=0)
    return out.reshape(B, C, N, 1).astype(np.float32)


LAST_RESULTS = None


# revision 10
# speedup vs baseline: 1.1077x; 1.0054x over previous
"""DG-block (dual graph-conv) Trainium2 kernel — nn_DG_Block.

Reference per batch item b (B=8, C=128, N=2000, K=9):
  idx1 = top9(knn keys on features_b); idx2 = top9(... motion_b)
  gf_i = graph_feature(features_b, idx_i) -> [2C, N, 9]
  f_i  = conv_bn_relu(1x3 stride 3) -> conv_bn_relu(1x3) on gf_i
  out_b = f1 + delta * f2        [C, N, 1]
BatchNorm pools over the WHOLE batch -> stats are all-reduced across cores.

Sharding: one batch item per NeuronCore (8 cores); conv/BN params replicated;
four [128,2] AllReduces reproduce the exact batch statistics.

Algebra (per branch; w1 [C,2C,1,3] split A_d = w1[:,:C,0,d], B_d = w1[:,C:,0,d]):
  conv1[o,n,t] = (P x_n)[o] - sum_d (B_d x_{idx[n,3t+d]})[o],  P = sum_d A_d+B_d
  (conv biases dropped: BN mean-subtraction cancels them exactly)
  rank-1 neighbor is the point itself -> folded into zA = (P - B_0) x for t=0.
  knn rank key: <x_i, x_j> - |x_j|^2/2  (fp32; monotone per-row transform of
  the reference's key). Diagonal mask -1e30 folded into the negsqh broadcast
  tile: the diag of chunk ci sits at col j with j = c0+p, i.e. j % 128 == p --
  ONE [C,N] tile masks every chunk.

Conv path runs in fp16 (tables, CCE gather-adds, PE transposes, conv2
operands); fp32 PSUM accumulation and fp32 BN statistics. Rel err ~5e-3.

Device pipeline per core:
  tables : yps [cn, 768] = x^T [ -B_0^T | -B_1^T | -B_2^T | zA^T | P^T | P^T ]
           (fp16 matmul); first 384 cols -> DRAM ytab (fp16), last 384 stay
           in SBUF as the conv1 accumulator init [zA|z|z].
  kNN    : pd chunk = X_chunk^T X on PE (fp32) ; DVE fuses PSUM drain +
           (-|x_j|^2/2 - 1e30*diag) add; DVE max8 + max_index.
  conv1  : per chunk THREE multi-offset indirect gathers (one per conv tap
           position d, 2-3 offsets per point) accumulate fp16 ytab rows onto
           the [zA|z|z] tile via SWDGE compute_op=add; per tap-group one fp16
           PE transpose -> stats on ACT (accum_out).
  conv2  : 3 accumulated fp16 matmuls; stats; AllReduce; final Relu-affines,
           f1 + delta*f2 on DVE, DMA out.
"""

import numpy as np

import concourse.bacc as bacc
import concourse.bass as bass
import concourse.mybir as mybir
import concourse.tile as tile
import concourse.bass_utils as bass_utils
from concourse.masks import make_identity

F32 = mybir.dt.float32
F32R = mybir.dt.float32r
F16 = mybir.dt.float16
U32 = mybir.dt.uint32
U16 = mybir.dt.uint16
I16 = mybir.dt.int16
AF = mybir.ActivationFunctionType
ALU = mybir.AluOpType

B = 8
C = 128
N = 2000
EPS = 1e-5
NEG_BIG = -1.0e30

CHUNKS = [(i * 128, min(128, N - i * 128)) for i in range((N + 127) // 128)]
NCH = len(CHUNKS)  # 16
# pd column tiles, 512-aligned so the diagonal block never straddles tiles
JT = [(j * 512, min(512, N - j * 512)) for j in range(4)]
WAVE = 4  # kNN/gather pipelining granularity (chunks)


def build_kernel(delta_nonneg: bool, pd_f32r: bool):
    # Calibrate the scheduler's SWDGE cost model to the measured DMAGatherAnt
    # rate (~7.5 ns/descriptor marginal vs the stock 0.34). The tile scheduler
    # plans engine orderings with this model; the stock value makes it pack
    # gathers 6x too tight and mis-order the queues. Must run before the
    # rust cost model's lazy hw-spec snapshot (OnceLock on first use).
    import concourse.hw_specs as hw_specs

    hw_specs.TRN2Spec.SWDGE_NS_PER_DESCRIPTOR = 7.5

    nc = bacc.Bacc(
        "TRN2",
        target_bir_lowering=False,
        debug=False,
        enable_asserts=False,
        num_devices=B,
    )

    feat_in = nc.dram_tensor("feat", [C, N], F32, kind="ExternalInput").ap()
    mot_in = nc.dram_tensor("mot", [C, N], F32, kind="ExternalInput").ap()
    wb = {}
    for br in (1, 2):
        wb[br] = {
            "nbt": nc.dram_tensor(f"nbt{br}", [C, 768], F16, kind="ExternalInput").ap(),
            "w2t": nc.dram_tensor(f"w2t{br}", [C, 3 * C], F16, kind="ExternalInput").ap(),
            "bn": nc.dram_tensor(f"bn{br}", [C, 4], F32, kind="ExternalInput").ap(),
        }
    delta_in = nc.dram_tensor("delta", [1, 1], F32, kind="ExternalInput").ap()
    out_t = nc.dram_tensor("out", [C, N], F32, kind="ExternalOutput").ap()

    with tile.TileContext(nc) as tc:
        _emit(nc, tc, feat_in, mot_in, wb, delta_in, out_t, delta_nonneg, pd_f32r)
    nc.compile()
    return nc


def _emit(nc, tc, feat_in, mot_in, wb, delta_in, out_t, delta_nonneg, pd_f32r):
    import contextlib

    ctx = contextlib.ExitStack()
    with ctx:
        sb = ctx.enter_context(tc.tile_pool(name="sb", bufs=1))
        pd_ps = ctx.enter_context(tc.tile_pool(name="pd_ps", bufs=2, space="PSUM"))
        st_ps = ctx.enter_context(tc.tile_pool(name="st_ps", bufs=2, space="PSUM"))
        o1_ps = ctx.enter_context(tc.tile_pool(name="o1_ps", bufs=2, space="PSUM"))
        dr = ctx.enter_context(tc.tile_pool(name="dr", bufs=1, space="DRAM"))

        # ---------------- persistent on-chip data ----------------
        x = sb.tile([C, N], F32, name="x")
        nc.sync.dma_start(out=x[:], in_=feat_in)
        m = sb.tile([C, N], F32, name="m")
        nc.sync.dma_start(out=m[:], in_=mot_in)
        xf16 = sb.tile([C, N], F16, name="xf16")
        nc.scalar.activation(out=xf16[:], in_=x[:], func=AF.Copy)

        ident = sb.tile([C, C], F32, name="ident")
        make_identity(nc, ident[:])
        ident16 = sb.tile([C, C], F16, name="ident16")
        nc.scalar.activation(out=ident16[:], in_=ident[:], func=AF.Copy)
        ineg = sb.tile([C, C], F32, name="ineg")
        nc.scalar.activation(out=ineg[:], in_=ident[:], func=AF.Copy, scale=NEG_BIG)
        ones1 = sb.tile([1, C], F32, name="ones1")
        nc.vector.memset(ones1[:], 1.0)
        neghalfc = sb.tile([C, 1], F32, name="neghalfc")
        nc.vector.memset(neghalfc[:], -0.5)

        w = {}
        for br in (1, 2):
            nbt = sb.tile([C, 768], F16, name=f"nbt{br}")
            nc.sync.dma_start(out=nbt[:], in_=wb[br]["nbt"])
            w2t = sb.tile([C, 3 * C], F16, name=f"w2t{br}")
            nc.sync.dma_start(out=w2t[:], in_=wb[br]["w2t"])
            bn = sb.tile([C, 4], F32, name=f"bn{br}")
            nc.sync.dma_start(out=bn[:], in_=wb[br]["bn"])
            w[br] = dict(nbt=nbt, w2t=w2t, bn=bn)

        delta_sb = sb.tile([1, 1], F32, name="delta_sb")
        nc.sync.dma_start(out=delta_sb[:], in_=delta_in)
        dps = st_ps.tile([C, 8], F32, name="dps", tag="stage")
        nc.tensor.matmul(
            out=dps[:, 0:1], lhsT=ones1[:], rhs=delta_sb[0:1, 0:1], start=True, stop=True
        )
        dcol = sb.tile([C, 1], F32, name="dcol")
        nc.scalar.activation(out=dcol[:], in_=dps[:, 0:1], func=AF.Copy)

        ytab = {br: dr.tile([N, 384], F16, name=f"ytab{br}") for br in (1, 2)}
        idx8 = {}
        for s in (1, 2):
            idx8[s] = sb.tile([C, NCH * 8], U16, name=f"idx8_{s}")
            nc.vector.memset(idx8[s][:], 0)
        # dma_gather infra: per conv-tap-position d (d=1 js{1,4,7}, d=2 js{2,5,8},
        # d=0 js{3,6}): NB_d blocks of 128 rows per branch. Gathered rows land at
        # out[g%128, g//128]; the int16 index list is "wrapped in 16 partitions,
        # replicated across cores": idxs16[q, s] = idx_of(g=16s+q).
        D_SPEC = ((1, 3, 0), (2, 3, 1), (0, 2, 2))  # (d, K_d, idx8-col-start)
        g8 = {
            (s, d): sb.tile([C, NCH * k * C], F16, name=f"g8_{s}_{d}")
            for d, k, _ in D_SPEC
            for s in (1, 2)
        }
        idxs16 = {
            (s, d): sb.tile([C, NCH * k * 8], U16, name=f"idxs16_{s}_{d}")
            for d, k, _ in D_SPEC
            for s in (1, 2)
        }

        # ---------------- tables ----------------
        g123 = {1: [], 2: []}

        def tables(br):
            for ci, (c0, cn) in enumerate(CHUNKS):
                yp1 = st_ps.tile([C, 384], F32, name=f"yp1_{br}_{ci}", tag="stage")
                nc.tensor.matmul(
                    out=yp1[:cn, :],
                    lhsT=xf16[:, c0 : c0 + cn],
                    rhs=w[br]["nbt"][:, 0:384],
                    start=True,
                    stop=True,
                )
                ytmp = sb.tile([C, 384], F16, name=f"ytmp_{br}_{ci}", tag="ytmp", bufs=3)
                nc.scalar.activation(out=ytmp[:cn, :], in_=yp1[:cn, :], func=AF.Copy)
                nc.sync.dma_start(out=ytab[br][c0 : c0 + cn, :], in_=ytmp[:cn, :])

                yp2 = st_ps.tile([C, 384], F32, name=f"yp2_{br}_{ci}", tag="stage")
                nc.tensor.matmul(
                    out=yp2[:cn, :],
                    lhsT=xf16[:, c0 : c0 + cn],
                    rhs=w[br]["nbt"][:, 384:768],
                    start=True,
                    stop=True,
                )
                gt = sb.tile([C, 384], F16, name=f"g_{br}_{ci}", tag=f"g{br}", bufs=NCH)
                nc.scalar.activation(out=gt[:cn, :], in_=yp2[:cn, :], func=AF.Copy)
                g123[br].append(gt)

        # ---------------- bulk neighbor gathers (dma_gather) ----------------
        def gather_half(br, which, half):
            """Gather neighbor rows for chunks [half*8, half*8+8) of a branch.

            Per conv-tap-position d: nb blocks of 128 rows; row g of the list
            lands at out[g%128, g//128]. idx list is int16, wrapped in 16
            partitions (idxs16[q, s] = idx_of(g=16s+q)), replicated across the
            8 Q7 cores. The wrap needs cross-partition motion -> DRAM bounce
            (interleave writes + replication readbacks) on the sync queue.
            """
            ch0 = half * (NCH // 2)
            chn = NCH // 2
            for d, k, s0 in D_SPEC:
                nb = chn * k
                perm = sb.tile(
                    [C, nb], U16, name=f"perm_{which}_{half}_{d}", tag="perm", bufs=3
                )
                nc.vector.tensor_scalar_add(
                    perm[:].rearrange("p (c j) -> p c j", c=chn),
                    idx8[which][:].rearrange("p (c j) -> p c j", c=NCH)[
                        :, ch0 : ch0 + chn, s0 : 8 : 3
                    ],
                    0,
                )
                scr = dr.tile([16, 8 * nb], U16, name=f"scr_{which}_{half}_{d}")
                for r in range(8):
                    ov = scr[:].rearrange("q (b r) -> q b r", r=8)[:, :, r].unsqueeze(2)
                    nc.sync.dma_start(
                        out=ov, in_=perm[16 * r : 16 * (r + 1), :].unsqueeze(2)
                    )
                ix = idxs16[(which, d)]
                col0 = ch0 * k * 8
                for kk in range(8):
                    nc.sync.dma_start(
                        out=ix[16 * kk : 16 * (kk + 1), col0 : col0 + nb * 8],
                        in_=scr[:],
                    )
                # HW ucode caps one instruction at 1024 indices (8 blocks)
                b0 = ch0 * k
                for i0 in range(0, nb, 8):
                    ib = min(8, nb - i0)
                    nc.gpsimd.dma_gather(
                        out_ap=g8[(which, d)][
                            :, (b0 + i0) * C : (b0 + i0 + ib) * C
                        ].rearrange("p (b n) -> p b n", n=C),
                        in_ap=ytab[br][:, d * C : (d + 1) * C],
                        idxs_ap=ix[:, (b0 + i0) * 8 : (b0 + i0 + ib) * 8].bitcast(I16),
                        num_idxs=ib * C,
                        num_idxs_reg=ib * C,
                        elem_size=C,
                        elem_step=384,
                    )

        def knn(src, which, br):
            # xsq = src*src (ACT); sqrow = -0.5 * colsum(xsq) (PE)
            xsq = sb.tile([C, N], F32, name=f"xsq_{which}", tag="xsq", bufs=1)
            nc.scalar.activation(out=xsq[:], in_=src[:], func=AF.Square)
            sqrow = sb.tile([1, N], F32, name=f"sqrow_{which}", tag="sqrow", bufs=1)
            for j0, jn in JT:
                sqps = st_ps.tile([1, 512], F32, name=f"sqps_{which}_{j0}", tag="stage")
                nc.tensor.matmul(
                    out=sqps[0:1, :jn],
                    lhsT=neghalfc[:],
                    rhs=xsq[:, j0 : j0 + jn],
                    start=True,
                    stop=True,
                )
                nc.scalar.activation(
                    out=sqrow[0:1, j0 : j0 + jn], in_=sqps[0:1, :jn], func=AF.Copy
                )
            # negsqh broadcast tile (DVE drains PSUM)
            nsd = sb.tile([C, N], F32, name=f"nsd_{which}", tag="nsd", bufs=1)
            for j0, jn in JT:
                nps = st_ps.tile([C, 512], F32, name=f"nps_{which}_{j0}", tag="stage")
                nc.tensor.matmul(
                    out=nps[:, :jn],
                    lhsT=ones1[:],
                    rhs=sqrow[0:1, j0 : j0 + jn],
                    start=True,
                    stop=True,
                )
                nc.scalar.activation(
                    out=nsd[:, j0 : j0 + jn], in_=nps[:, :jn], func=AF.Copy
                )

            halves = [(0, [JT[0], JT[1]]), (1024, [JT[2], JT[3]])]
            if True:
                for ci, (c0, cn) in enumerate(CHUNKS):
                    if ci == NCH // 2:
                        gather_half(br, which, 0)
                    pdt = sb.tile([C, N], F32, name=f"pd_{which}_{ci}", tag="pd", bufs=2)
                    for h0, jts in halves:
                        pps = pd_ps.tile(
                            [C, 1024], F32, name=f"pps_{which}_{ci}_{h0}", tag="pdps"
                        )
                        off = 0
                        for j0, jn in jts:
                            if pd_f32r:
                                nc.tensor.matmul(
                                    out=pps[:cn, off : off + jn],
                                    lhsT=src[:, c0 : c0 + cn].bitcast(F32R),
                                    rhs=src[:, j0 : j0 + jn].bitcast(F32R),
                                    start=True,
                                    stop=True,
                                )
                            else:
                                nc.tensor.matmul(
                                    out=pps[:cn, off : off + jn],
                                    lhsT=src[:, c0 : c0 + cn],
                                    rhs=src[:, j0 : j0 + jn],
                                    start=True,
                                    stop=True,
                                )
                            off += jn
                        nc.vector.tensor_tensor(
                            out=pdt[:cn, h0 : h0 + off],
                            in0=pps[:cn, 0:off],
                            in1=nsd[:cn, h0 : h0 + off],
                            op=ALU.add,
                        )
                    # mask the current chunk's diagonal block
                    nc.vector.tensor_tensor(
                        out=pdt[:cn, c0 : c0 + cn],
                        in0=pdt[:cn, c0 : c0 + cn],
                        in1=ineg[:cn, :cn],
                        op=ALU.add,
                    )
                    vals8 = sb.tile([C, 8], F32, name=f"v8_{which}_{ci}", tag="v8", bufs=2)
                    nc.vector.max(out=vals8[:cn], in_=pdt[:cn, :])
                    nc.vector.max_index(
                        out=idx8[which][:cn, ci * 8 : ci * 8 + 8],
                        in_max=vals8[:cn],
                        in_values=pdt[:cn, :],
                    )
                gather_half(br, which, 1)

        # ---------------- conv1 transposes + stats ----------------
        o1_tiles = {}
        stats1 = {}

        def tap_adds(br):
            # fold gathered neighbor rows onto the [zA|z|z] accumulators
            # (fp16 SBUF operands -> DVE 4x mode)
            for ci, (c0, cn) in enumerate(CHUNKS):
                gt = g123[br][ci]
                for d, k, _ in D_SPEC:
                    col0 = 384 - k * C
                    nc.vector.tensor_tensor(
                        out=gt[:cn, col0:384],
                        in0=gt[:cn, col0:384],
                        in1=g8[(br, d)][:cn, ci * k * C : (ci + 1) * k * C],
                        op=ALU.add,
                    )

        def conv1_t(br):
            ol = []
            s1 = sb.tile([C, NCH], F32, name=f"s1c_{br}")
            s2 = sb.tile([C, NCH], F32, name=f"s2c_{br}")
            for ci, (c0, cn) in enumerate(CHUNKS):
                ops = o1_ps.tile([C, 384], F16, name=f"o1ps_{br}_{ci}", tag="o1")
                for t in range(3):
                    nc.tensor.matmul(
                        out=ops[:, t * C : t * C + cn],
                        lhsT=g123[br][ci][:cn, t * C : (t + 1) * C],
                        rhs=ident16[:cn, :cn],
                        is_transpose=True,
                        start=True,
                        stop=True,
                        skip_group_check=True,
                    )
                src_ap = ops[:, 0:384].rearrange("p (t n) -> p t n", t=3)[:, :, :cn]
                ot = sb.tile([C, 384], F16, name=f"o1_{br}_{ci}", tag=f"o1{br}", bufs=NCH)
                dst_ap = ot[:, 0:384].rearrange("p (t n) -> p t n", t=3)[:, :, :cn]
                nc.scalar.activation(
                    out=dst_ap, in_=src_ap, func=AF.Copy, accum_out=s1[:, ci : ci + 1]
                )
                osq = sb.tile([C, 384], F16, name=f"o1sq_{br}_{ci}", tag="o1sq", bufs=2)
                sq_ap = osq[:, 0:384].rearrange("p (t n) -> p t n", t=3)[:, :, :cn]
                nc.scalar.activation(
                    out=sq_ap, in_=src_ap, func=AF.Square, accum_out=s2[:, ci : ci + 1]
                )
                ol.append(ot)
            o1_tiles[br] = ol
            stats1[br] = (s1, s2)

        # ---------------- allreduce + affine computation ----------------
        def allreduce_affine(stats_br, m_count, bn_cols, round_id, br):
            s1, s2 = stats_br
            arq = sb.tile([C, 2], F32, name=f"arq{round_id}")
            nc.vector.reduce_sum(out=arq[:, 0:1], in_=s1[:], axis=mybir.AxisListType.X)
            nc.vector.reduce_sum(out=arq[:, 1:2], in_=s2[:], axis=mybir.AxisListType.X)
            ar_in = dr.tile([C, 2], F32, name=f"arin{round_id}")
            ar_out = dr.tile([C, 2], F32, name=f"arout{round_id}", addr_space="Shared")
            nc.sync.dma_start(out=ar_in[:], in_=arq[:])
            nc.gpsimd.collective_compute(
                "AllReduce",
                ALU.add,
                replica_groups=[list(range(B))],
                ins=[ar_in[:].opt()],
                outs=[ar_out[:].opt()],
            )
            art = sb.tile([C, 2], F32, name=f"art{round_id}")
            nc.sync.dma_start(out=art[:], in_=ar_out[:])

            inv_m = 1.0 / float(m_count)
            gcol = w[br]["bn"][:, bn_cols[0] : bn_cols[0] + 1]
            bcol = w[br]["bn"][:, bn_cols[1] : bn_cols[1] + 1]
            mean = sb.tile([C, 1], F32, name=f"mean{round_id}_{br}")
            nc.vector.tensor_scalar_mul(mean[:], art[:, 0:1], inv_m)
            ey2 = sb.tile([C, 1], F32, name=f"ey2{round_id}_{br}")
            nc.vector.tensor_scalar_mul(ey2[:], art[:, 1:2], inv_m)
            var = sb.tile([C, 1], F32, name=f"var{round_id}_{br}")
            nc.vector.tensor_tensor(out=var[:], in0=mean[:], in1=mean[:], op=ALU.mult)
            nc.vector.tensor_tensor(out=var[:], in0=ey2[:], in1=var[:], op=ALU.subtract)
            nc.vector.tensor_scalar_add(var[:], var[:], EPS)
            rv = sb.tile([C, 1], F32, name=f"rv{round_id}_{br}")
            nc.vector.reciprocal(rv[:], var[:])
            rstd = sb.tile([C, 1], F32, name=f"rstd{round_id}_{br}")
            nc.scalar.activation(out=rstd[:], in_=rv[:], func=AF.Sqrt)
            a_col = sb.tile([C, 1], F32, name=f"acol{round_id}_{br}")
            nc.vector.tensor_tensor(out=a_col[:], in0=gcol, in1=rstd[:], op=ALU.mult)
            c_col = sb.tile([C, 1], F32, name=f"ccol{round_id}_{br}")
            nc.vector.tensor_tensor(out=c_col[:], in0=mean[:], in1=a_col[:], op=ALU.mult)
            nc.vector.tensor_tensor(out=c_col[:], in0=bcol, in1=c_col[:], op=ALU.subtract)
            return (a_col, c_col)

        # ---------------- conv2 + stats ----------------
        o2_tiles = {}
        stats2 = {}

        def conv2(br, aff):
            a_col, c_col = aff
            ol = []
            s1 = sb.tile([C, NCH], F32, name=f"s1d_{br}")
            s2 = sb.tile([C, NCH], F32, name=f"s2d_{br}")
            for ci, (c0, cn) in enumerate(CHUNKS):
                ot = o1_tiles[br][ci]
                o1r_ap = ot[:, 0:384].rearrange("p (t n) -> p t n", t=3)[:, :, :cn]
                nc.scalar.activation(
                    out=o1r_ap, in_=o1r_ap, func=AF.Relu, scale=a_col[:], bias=c_col[:]
                )
                o2ps = st_ps.tile([C, 128], F32, name=f"o2ps_{br}_{ci}", tag="stage")
                for d in range(3):
                    nc.tensor.matmul(
                        out=o2ps[:, :cn],
                        lhsT=w[br]["w2t"][:, d * C : (d + 1) * C],
                        rhs=ot[:, d * C : d * C + cn],
                        start=(d == 0),
                        stop=(d == 2),
                    )
                o2 = sb.tile([C, C], F32, name=f"o2_{br}_{ci}", tag=f"o2{br}", bufs=NCH)
                nc.scalar.activation(
                    out=o2[:, :cn],
                    in_=o2ps[:, :cn],
                    func=AF.Copy,
                    accum_out=s1[:, ci : ci + 1],
                )
                osq = sb.tile([C, C], F32, name=f"o2sq_{br}_{ci}", tag="o2sq", bufs=2)
                nc.scalar.activation(
                    out=osq[:, :cn],
                    in_=o2ps[:, :cn],
                    func=AF.Square,
                    accum_out=s2[:, ci : ci + 1],
                )
                ol.append(o2)
            o2_tiles[br] = ol
            stats2[br] = (s1, s2)

        # ---------------- emit ----------------
        tables(1)
        knn(x, 1, 1)
        tables(2)
        knn(m, 2, 2)

        tap_adds(1)
        conv1_t(1)
        tap_adds(2)
        conv1_t(2)

        aff1_1 = allreduce_affine(stats1[1], B * N * 3, (0, 1), "1a", 1)
        conv2(1, aff1_1)
        aff1_2 = allreduce_affine(stats1[2], B * N * 3, (0, 1), "1b", 2)
        conv2(2, aff1_2)

        aff2_1 = allreduce_affine(stats2[1], B * N, (2, 3), "2a", 1)
        a1, c1 = aff2_1
        f1_tiles = []
        for ci, (c0, cn) in enumerate(CHUNKS):
            f1t = o2_tiles[1][ci]
            nc.scalar.activation(
                out=f1t[:, :cn],
                in_=f1t[:, :cn],
                func=AF.Relu,
                scale=a1[:],
                bias=c1[:],
            )
            f1_tiles.append(f1t)

        aff2_2 = allreduce_affine(stats2[2], B * N, (2, 3), "2b", 2)
        a2, c2 = aff2_2
        if delta_nonneg:
            a2d = sb.tile([C, 1], F32, name="a2d")
            nc.vector.tensor_tensor(out=a2d[:], in0=a2[:], in1=dcol[:], op=ALU.mult)
            c2d = sb.tile([C, 1], F32, name="c2d")
            nc.vector.tensor_tensor(out=c2d[:], in0=c2[:], in1=dcol[:], op=ALU.mult)
        for ci, (c0, cn) in enumerate(CHUNKS):
            f1t = f1_tiles[ci]
            f2t = sb.tile([C, C], F32, name=f"f2_{ci}", tag="f2", bufs=2)
            if delta_nonneg:
                nc.scalar.activation(
                    out=f2t[:, :cn],
                    in_=o2_tiles[2][ci][:, :cn],
                    func=AF.Relu,
                    scale=a2d[:],
                    bias=c2d[:],
                )
            else:
                nc.scalar.activation(
                    out=f2t[:, :cn],
                    in_=o2_tiles[2][ci][:, :cn],
                    func=AF.Relu,
                    scale=a2[:],
                    bias=c2[:],
                )
                nc.vector.tensor_scalar_mul(f2t[:, :cn], f2t[:, :cn], dcol[:])
            of = sb.tile([C, C], F32, name=f"of_{ci}", tag="of", bufs=2)
            nc.vector.tensor_tensor(
                out=of[:, :cn], in0=f1t[:, :cn], in1=f2t[:, :cn], op=ALU.add
            )
            nc.sync.dma_start(out=out_t[:, c0 : c0 + cn], in_=of[:, :cn])


# ======================= host side =======================

_CACHE = {}


def _prep_branch(w1, b1, g1, be1, w2, b2, g2, be2):
    w1 = np.asarray(w1, dtype=np.float32)
    w2 = np.asarray(w2, dtype=np.float32)
    A = w1[:, :C, 0, :]  # [o, i, 3]
    Bm = w1[:, C:, 0, :]  # [o, i, 3]
    P = (A + Bm).sum(axis=2)  # [o, i]
    blocks = [np.ascontiguousarray((-Bm[:, :, d]).T) for d in range(3)]  # [i, o]
    zA = np.ascontiguousarray((P - Bm[:, :, 0]).T)
    z = np.ascontiguousarray(P.T)
    nbt_ext = np.concatenate(blocks + [zA, z, z], axis=1).astype(np.float16)  # [C,768]
    w2t = np.ascontiguousarray(
        np.concatenate([w2[:, :, 0, d].T for d in range(3)], axis=1)
    ).astype(np.float16)  # [C, 3C]
    bn = np.ascontiguousarray(
        np.stack(
            [
                np.asarray(g1, np.float32),
                np.asarray(be1, np.float32),
                np.asarray(g2, np.float32),
                np.asarray(be2, np.float32),
            ],
            axis=1,
        )
    )  # [C, 4]
    return nbt_ext, w2t, bn


def kernel(**inputs):
    import os

    features = np.ascontiguousarray(np.asarray(inputs["features"], np.float32))
    motion = np.ascontiguousarray(np.asarray(inputs["motion"], np.float32))
    delta = np.asarray(inputs["delta"], np.float32).reshape(-1)[0]

    nbt1, w2t1, bn1 = _prep_branch(
        inputs["d1_w1"], inputs["d1_b1"], inputs["d1_g1"], inputs["d1_be1"],
        inputs["d1_w2"], inputs["d1_b2"], inputs["d1_g2"], inputs["d1_be2"],
    )
    nbt2, w2t2, bn2 = _prep_branch(
        inputs["d2_w1"], inputs["d2_b1"], inputs["d2_g1"], inputs["d2_be1"],
        inputs["d2_w2"], inputs["d2_b2"], inputs["d2_g2"], inputs["d2_be2"],
    )

    delta_nonneg = bool(delta >= 0.0)
    pd_f32r = bool(int(os.environ.get("DG_PD_F32R", "0")))
    key = ("dg", delta_nonneg, pd_f32r)
    if key not in _CACHE:
        _CACHE[key] = build_kernel(delta_nonneg, pd_f32r)
    nc = _CACHE[key]

    shared = {
        "nbt1": nbt1, "w2t1": w2t1, "bn1": bn1,
        "nbt2": nbt2, "w2t2": w2t2, "bn2": bn2,
        "delta": np.array([[delta]], np.float32),
    }
    in_maps = []
    for c in range(B):
        im = dict(shared)
        im["feat"] = np.ascontiguousarray(features[c, :, :, 0])
        im["mot"] = np.ascontiguousarray(motion[c, :, :, 0])
        in_maps.append(im)

    trace = bool(int(os.environ.get("DG_KERNEL_TRACE", "0")))
    res = bass_utils.run_bass_kernel_spmd(
        nc, in_maps, core_ids=list(range(B)), trace=trace
    )
    global LAST_RESULTS
    LAST_RESULTS = res
    out = np.stack([res.results[c]["out"] for c in range(B)], axis=0)
    return out.reshape(B, C, N, 1).astype(np.float32)


LAST_RESULTS = None


# revision 11
# speedup vs baseline: 1.1746x; 1.0604x over previous
"""DG-block (dual graph-conv) Trainium2 kernel — nn_DG_Block.

Reference per batch item b (B=8, C=128, N=2000, K=9):
  idx1 = top9(knn keys on features_b); idx2 = top9(... motion_b)
  gf_i = graph_feature(features_b, idx_i) -> [2C, N, 9]
  f_i  = conv_bn_relu(1x3 stride 3) -> conv_bn_relu(1x3) on gf_i
  out_b = f1 + delta * f2        [C, N, 1]
BatchNorm pools over the WHOLE batch -> stats are all-reduced across cores.

Sharding: one batch item per NeuronCore (8 cores); conv/BN params replicated;
four [128,2] AllReduces reproduce the exact batch statistics.

Algebra (per branch; w1 [C,2C,1,3] split A_d = w1[:,:C,0,d], B_d = w1[:,C:,0,d]):
  conv1[o,n,t] = (P x_n)[o] - sum_d (B_d x_{idx[n,3t+d]})[o],  P = sum_d A_d+B_d
  (conv biases dropped: BN mean-subtraction cancels them exactly)
  rank-1 neighbor is the point itself -> folded into zA = (P - B_0) x for t=0.
  knn rank key: <x_i, x_j> - |x_j|^2/2  (fp32; monotone per-row transform of
  the reference's key). Diagonal mask -1e30 folded into the negsqh broadcast
  tile: the diag of chunk ci sits at col j with j = c0+p, i.e. j % 128 == p --
  ONE [C,N] tile masks every chunk.

Conv path runs in fp16 (tables, CCE gather-adds, PE transposes, conv2
operands); fp32 PSUM accumulation and fp32 BN statistics. Rel err ~5e-3.

Device pipeline per core:
  tables : yps [cn, 768] = x^T [ -B_0^T | -B_1^T | -B_2^T | zA^T | P^T | P^T ]
           (fp16 matmul); first 384 cols -> DRAM ytab (fp16), last 384 stay
           in SBUF as the conv1 accumulator init [zA|z|z].
  kNN    : pd chunk = X_chunk^T X on PE (fp32) ; DVE fuses PSUM drain +
           (-|x_j|^2/2 - 1e30*diag) add; DVE max8 + max_index.
  conv1  : per chunk THREE multi-offset indirect gathers (one per conv tap
           position d, 2-3 offsets per point) accumulate fp16 ytab rows onto
           the [zA|z|z] tile via SWDGE compute_op=add; per tap-group one fp16
           PE transpose -> stats on ACT (accum_out).
  conv2  : 3 accumulated fp16 matmuls; stats; AllReduce; final Relu-affines,
           f1 + delta*f2 on DVE, DMA out.
"""

import numpy as np

import concourse.bacc as bacc
import concourse.bass as bass
import concourse.mybir as mybir
import concourse.tile as tile
import concourse.bass_utils as bass_utils
from concourse.masks import make_identity

F32 = mybir.dt.float32
F32R = mybir.dt.float32r
F16 = mybir.dt.float16
U32 = mybir.dt.uint32
U16 = mybir.dt.uint16
I16 = mybir.dt.int16
AF = mybir.ActivationFunctionType
ALU = mybir.AluOpType

B = 8
C = 128
N = 2000
EPS = 1e-5
NEG_BIG = -1.0e30

CHUNKS = [(i * 128, min(128, N - i * 128)) for i in range((N + 127) // 128)]
NCH = len(CHUNKS)  # 16
# pd column tiles, 512-aligned so the diagonal block never straddles tiles
JT = [(j * 512, min(512, N - j * 512)) for j in range(4)]
WAVE = 4  # kNN/gather pipelining granularity (chunks)


def build_kernel(delta_nonneg: bool, pd_f32r: bool):
    # Calibrate the scheduler's SWDGE cost model to the measured DMAGatherAnt
    # rate (~7.5 ns/descriptor marginal vs the stock 0.34). The tile scheduler
    # plans engine orderings with this model; the stock value makes it pack
    # gathers 6x too tight and mis-order the queues. Must run before the
    # rust cost model's lazy hw-spec snapshot (OnceLock on first use).
    import concourse.hw_specs as hw_specs

    hw_specs.TRN2Spec.SWDGE_NS_PER_DESCRIPTOR = 7.5

    nc = bacc.Bacc(
        "TRN2",
        target_bir_lowering=False,
        debug=False,
        enable_asserts=False,
        num_devices=B,
    )

    feat_in = nc.dram_tensor("feat", [C, N], F32, kind="ExternalInput").ap()
    mot_in = nc.dram_tensor("mot", [C, N], F32, kind="ExternalInput").ap()
    wb = {}
    for br in (1, 2):
        wb[br] = {
            "nbt": nc.dram_tensor(f"nbt{br}", [C, 768], F16, kind="ExternalInput").ap(),
            "w2t": nc.dram_tensor(f"w2t{br}", [C, 3 * C], F16, kind="ExternalInput").ap(),
            "bn": nc.dram_tensor(f"bn{br}", [C, 4], F32, kind="ExternalInput").ap(),
        }
    delta_in = nc.dram_tensor("delta", [1, 1], F32, kind="ExternalInput").ap()
    out_t = nc.dram_tensor("out", [C, N], F32, kind="ExternalOutput").ap()

    with tile.TileContext(nc) as tc:
        _emit(nc, tc, feat_in, mot_in, wb, delta_in, out_t, delta_nonneg, pd_f32r)
    nc.compile()
    return nc


def _emit(nc, tc, feat_in, mot_in, wb, delta_in, out_t, delta_nonneg, pd_f32r):
    import contextlib

    ctx = contextlib.ExitStack()
    with ctx:
        sb = ctx.enter_context(tc.tile_pool(name="sb", bufs=1))
        pd_ps = ctx.enter_context(tc.tile_pool(name="pd_ps", bufs=2, space="PSUM"))
        st_ps = ctx.enter_context(tc.tile_pool(name="st_ps", bufs=2, space="PSUM"))
        o1_ps = ctx.enter_context(tc.tile_pool(name="o1_ps", bufs=2, space="PSUM"))
        dr = ctx.enter_context(tc.tile_pool(name="dr", bufs=1, space="DRAM"))

        # ---------------- persistent on-chip data ----------------
        x = sb.tile([C, N], F32, name="x")
        nc.sync.dma_start(out=x[:], in_=feat_in)
        m = sb.tile([C, N], F32, name="m")
        nc.sync.dma_start(out=m[:], in_=mot_in)
        xf16 = sb.tile([C, N], F16, name="xf16")
        nc.scalar.activation(out=xf16[:], in_=x[:], func=AF.Copy)

        ident = sb.tile([C, C], F32, name="ident")
        make_identity(nc, ident[:])
        ident16 = sb.tile([C, C], F16, name="ident16")
        nc.scalar.activation(out=ident16[:], in_=ident[:], func=AF.Copy)
        ineg = sb.tile([C, C], F32, name="ineg")
        nc.scalar.activation(out=ineg[:], in_=ident[:], func=AF.Copy, scale=NEG_BIG)
        ones1 = sb.tile([1, C], F32, name="ones1")
        nc.vector.memset(ones1[:], 1.0)
        neghalfc = sb.tile([C, 1], F32, name="neghalfc")
        nc.vector.memset(neghalfc[:], -0.5)

        w = {}
        for br in (1, 2):
            nbt = sb.tile([C, 768], F16, name=f"nbt{br}")
            nc.sync.dma_start(out=nbt[:], in_=wb[br]["nbt"])
            w2t = sb.tile([C, 3 * C], F16, name=f"w2t{br}")
            nc.sync.dma_start(out=w2t[:], in_=wb[br]["w2t"])
            bn = sb.tile([C, 4], F32, name=f"bn{br}")
            nc.sync.dma_start(out=bn[:], in_=wb[br]["bn"])
            w[br] = dict(nbt=nbt, w2t=w2t, bn=bn)

        delta_sb = sb.tile([1, 1], F32, name="delta_sb")
        nc.sync.dma_start(out=delta_sb[:], in_=delta_in)
        dps = st_ps.tile([C, 8], F32, name="dps", tag="stage")
        nc.tensor.matmul(
            out=dps[:, 0:1], lhsT=ones1[:], rhs=delta_sb[0:1, 0:1], start=True, stop=True
        )
        dcol = sb.tile([C, 1], F32, name="dcol")
        nc.scalar.activation(out=dcol[:], in_=dps[:, 0:1], func=AF.Copy)

        ytab = {br: dr.tile([N, 384], F16, name=f"ytab{br}") for br in (1, 2)}
        idx8 = {}
        for s in (1, 2):
            idx8[s] = sb.tile([C, NCH * 8], U16, name=f"idx8_{s}")
            nc.vector.memset(idx8[s][:], 0)
        # dma_gather infra: per conv-tap-position d (d=1 js{1,4,7}, d=2 js{2,5,8},
        # d=0 js{3,6}): NB_d blocks of 128 rows per branch. Gathered rows land at
        # out[g%128, g//128]; the int16 index list is "wrapped in 16 partitions,
        # replicated across cores": idxs16[q, s] = idx_of(g=16s+q).
        D_SPEC = ((1, 3, 0), (2, 3, 1), (0, 2, 2))  # (d, K_d, idx8-col-start)
        g8 = {
            (s, d): sb.tile([C, NCH * k * C], F16, name=f"g8_{s}_{d}")
            for d, k, _ in D_SPEC
            for s in (1, 2)
        }
        idxs16 = {
            (s, h): sb.tile([C, (NCH // 2) * 8 * 8], U16, name=f"idxs16_{s}_{h}")
            for h in (0, 1)
            for s in (1, 2)
        }

        # ---------------- tables ----------------
        g123 = {1: [], 2: []}

        def tables(br):
            for ci, (c0, cn) in enumerate(CHUNKS):
                yp1 = st_ps.tile([C, 384], F32, name=f"yp1_{br}_{ci}", tag="stage")
                nc.tensor.matmul(
                    out=yp1[:cn, :],
                    lhsT=xf16[:, c0 : c0 + cn],
                    rhs=w[br]["nbt"][:, 0:384],
                    start=True,
                    stop=True,
                )
                ytmp = sb.tile([C, 384], F16, name=f"ytmp_{br}_{ci}", tag="ytmp", bufs=3)
                nc.scalar.activation(out=ytmp[:cn, :], in_=yp1[:cn, :], func=AF.Copy)
                nc.sync.dma_start(out=ytab[br][c0 : c0 + cn, :], in_=ytmp[:cn, :])

                yp2 = st_ps.tile([C, 384], F32, name=f"yp2_{br}_{ci}", tag="stage")
                nc.tensor.matmul(
                    out=yp2[:cn, :],
                    lhsT=xf16[:, c0 : c0 + cn],
                    rhs=w[br]["nbt"][:, 384:768],
                    start=True,
                    stop=True,
                )
                gt = sb.tile([C, 384], F16, name=f"g_{br}_{ci}", tag=f"g{br}", bufs=NCH)
                nc.scalar.activation(out=gt[:cn, :], in_=yp2[:cn, :], func=AF.Copy)
                g123[br].append(gt)

        # ---------------- bulk neighbor gathers (dma_gather) ----------------
        def gather_half(br, which, half):
            """Gather neighbor rows for chunks [half*8, half*8+8) of a branch.

            Per conv-tap-position d: blocks of 128 rows; row g of a gather
            list lands at out[g%128, g//128]. idx lists are int16, wrapped in
            16 partitions (idxs16[q, s] = idx_of(g=16s+q)), replicated across
            the 8 Q7 cores. The wrap needs cross-partition motion -> one DRAM
            bounce per half (8 interleave writes + 8 replication readbacks)
            covering all three d-groups at once.
            """
            ch0 = half * (NCH // 2)
            chn = NCH // 2
            ncols = chn * 8  # 64 combined index columns per half
            with tc.high_priority():
                perm = sb.tile(
                    [C, ncols], U16, name=f"perm_{which}_{half}", tag="perm", bufs=2
                )
                coff = 0
                for d, k, s0 in D_SPEC:
                    nb = chn * k
                    nc.vector.tensor_scalar_add(
                        perm[:, coff : coff + nb].rearrange("p (c j) -> p c j", c=chn),
                        idx8[which][:].rearrange("p (c j) -> p c j", c=NCH)[
                            :, ch0 : ch0 + chn, s0 : 8 : 3
                        ],
                        0,
                    )
                    coff += nb
                scr = dr.tile([16, 8 * ncols], U16, name=f"scr_{which}_{half}")
                for r in range(8):
                    ov = scr[:].rearrange("q (b r) -> q b r", r=8)[:, :, r].unsqueeze(2)
                    nc.sync.dma_start(
                        out=ov, in_=perm[16 * r : 16 * (r + 1), :].unsqueeze(2)
                    )
                ixa = idxs16[(which, half)]
                for kk in range(8):
                    nc.sync.dma_start(
                        out=ixa[16 * kk : 16 * (kk + 1), :], in_=scr[:]
                    )
                coff = 0
                for d, k, s0 in D_SPEC:
                    nb = chn * k
                    b0 = ch0 * k
                    # HW ucode caps one instruction at 1024 indices (8 blocks)
                    for i0 in range(0, nb, 8):
                        ib = min(8, nb - i0)
                        nc.gpsimd.dma_gather(
                            out_ap=g8[(which, d)][
                                :, (b0 + i0) * C : (b0 + i0 + ib) * C
                            ].rearrange("p (b n) -> p b n", n=C),
                            in_ap=ytab[br][:, d * C : (d + 1) * C],
                            idxs_ap=ixa[
                                :, (coff + i0) * 8 : (coff + i0 + ib) * 8
                            ].bitcast(I16),
                            num_idxs=ib * C,
                            num_idxs_reg=ib * C,
                            elem_size=C,
                            elem_step=384,
                        )
                    coff += nb

        def knn(src, which, br):
            # xsq = src*src (ACT); sqrow = -0.5 * colsum(xsq) (PE)
            xsq = sb.tile([C, N], F32, name=f"xsq_{which}", tag="xsq", bufs=1)
            nc.scalar.activation(out=xsq[:], in_=src[:], func=AF.Square)
            sqrow = sb.tile([1, N], F32, name=f"sqrow_{which}", tag="sqrow", bufs=1)
            for j0, jn in JT:
                sqps = st_ps.tile([1, 512], F32, name=f"sqps_{which}_{j0}", tag="stage")
                nc.tensor.matmul(
                    out=sqps[0:1, :jn],
                    lhsT=neghalfc[:],
                    rhs=xsq[:, j0 : j0 + jn],
                    start=True,
                    stop=True,
                )
                nc.scalar.activation(
                    out=sqrow[0:1, j0 : j0 + jn], in_=sqps[0:1, :jn], func=AF.Copy
                )
            # negsqh broadcast tile (DVE drains PSUM)
            nsd = sb.tile([C, N], F32, name=f"nsd_{which}", tag="nsd", bufs=1)
            for j0, jn in JT:
                nps = st_ps.tile([C, 512], F32, name=f"nps_{which}_{j0}", tag="stage")
                nc.tensor.matmul(
                    out=nps[:, :jn],
                    lhsT=ones1[:],
                    rhs=sqrow[0:1, j0 : j0 + jn],
                    start=True,
                    stop=True,
                )
                nc.scalar.activation(
                    out=nsd[:, j0 : j0 + jn], in_=nps[:, :jn], func=AF.Copy
                )

            halves = [(0, [JT[0], JT[1]]), (1024, [JT[2], JT[3]])]
            if True:
                for ci, (c0, cn) in enumerate(CHUNKS):
                    if ci == NCH // 2:
                        gather_half(br, which, 0)
                    pdt = sb.tile([C, N], F32, name=f"pd_{which}_{ci}", tag="pd", bufs=2)
                    for h0, jts in halves:
                        pps = pd_ps.tile(
                            [C, 1024], F32, name=f"pps_{which}_{ci}_{h0}", tag="pdps"
                        )
                        off = 0
                        for j0, jn in jts:
                            if pd_f32r:
                                nc.tensor.matmul(
                                    out=pps[:cn, off : off + jn],
                                    lhsT=src[:, c0 : c0 + cn].bitcast(F32R),
                                    rhs=src[:, j0 : j0 + jn].bitcast(F32R),
                                    start=True,
                                    stop=True,
                                )
                            else:
                                nc.tensor.matmul(
                                    out=pps[:cn, off : off + jn],
                                    lhsT=src[:, c0 : c0 + cn],
                                    rhs=src[:, j0 : j0 + jn],
                                    start=True,
                                    stop=True,
                                )
                            off += jn
                        nc.vector.tensor_tensor(
                            out=pdt[:cn, h0 : h0 + off],
                            in0=pps[:cn, 0:off],
                            in1=nsd[:cn, h0 : h0 + off],
                            op=ALU.add,
                        )
                    # mask the current chunk's diagonal block
                    nc.vector.tensor_tensor(
                        out=pdt[:cn, c0 : c0 + cn],
                        in0=pdt[:cn, c0 : c0 + cn],
                        in1=ineg[:cn, :cn],
                        op=ALU.add,
                    )
                    vals8 = sb.tile([C, 8], F32, name=f"v8_{which}_{ci}", tag="v8", bufs=2)
                    nc.vector.max(out=vals8[:cn], in_=pdt[:cn, :])
                    nc.vector.max_index(
                        out=idx8[which][:cn, ci * 8 : ci * 8 + 8],
                        in_max=vals8[:cn],
                        in_values=pdt[:cn, :],
                    )
                gather_half(br, which, 1)

        # ---------------- conv1 transposes + stats ----------------
        o1_tiles = {}
        stats1 = {}

        def tap_adds(br):
            # fold gathered neighbor rows onto the [zA|z|z] accumulators
            # (fp16 SBUF operands -> DVE 4x mode)
            for ci, (c0, cn) in enumerate(CHUNKS):
                gt = g123[br][ci]
                for d, k, _ in D_SPEC:
                    col0 = 384 - k * C
                    nc.vector.tensor_tensor(
                        out=gt[:cn, col0:384],
                        in0=gt[:cn, col0:384],
                        in1=g8[(br, d)][:cn, ci * k * C : (ci + 1) * k * C],
                        op=ALU.add,
                    )

        def conv1_t(br):
            ol = []
            s1 = sb.tile([C, NCH], F32, name=f"s1c_{br}")
            s2 = sb.tile([C, NCH], F32, name=f"s2c_{br}")
            for ci, (c0, cn) in enumerate(CHUNKS):
                ops = o1_ps.tile([C, 384], F16, name=f"o1ps_{br}_{ci}", tag="o1")
                for t in range(3):
                    nc.tensor.matmul(
                        out=ops[:, t * C : t * C + cn],
                        lhsT=g123[br][ci][:cn, t * C : (t + 1) * C],
                        rhs=ident16[:cn, :cn],
                        is_transpose=True,
                        start=True,
                        stop=True,
                        skip_group_check=True,
                    )
                src_ap = ops[:, 0:384].rearrange("p (t n) -> p t n", t=3)[:, :, :cn]
                ot = sb.tile([C, 384], F16, name=f"o1_{br}_{ci}", tag=f"o1{br}", bufs=NCH)
                dst_ap = ot[:, 0:384].rearrange("p (t n) -> p t n", t=3)[:, :, :cn]
                nc.scalar.activation(
                    out=dst_ap, in_=src_ap, func=AF.Copy, accum_out=s1[:, ci : ci + 1]
                )
                osq = sb.tile([C, 384], F16, name=f"o1sq_{br}_{ci}", tag="o1sq", bufs=2)
                sq_ap = osq[:, 0:384].rearrange("p (t n) -> p t n", t=3)[:, :, :cn]
                nc.scalar.activation(
                    out=sq_ap, in_=src_ap, func=AF.Square, accum_out=s2[:, ci : ci + 1]
                )
                ol.append(ot)
            o1_tiles[br] = ol
            stats1[br] = (s1, s2)

        # ---------------- allreduce + affine computation ----------------
        def allreduce_affine(stats_br, m_count, bn_cols, round_id, br):
            s1, s2 = stats_br
            arq = sb.tile([C, 2], F32, name=f"arq{round_id}")
            nc.vector.reduce_sum(out=arq[:, 0:1], in_=s1[:], axis=mybir.AxisListType.X)
            nc.vector.reduce_sum(out=arq[:, 1:2], in_=s2[:], axis=mybir.AxisListType.X)
            ar_in = dr.tile([C, 2], F32, name=f"arin{round_id}")
            ar_out = dr.tile([C, 2], F32, name=f"arout{round_id}", addr_space="Shared")
            nc.sync.dma_start(out=ar_in[:], in_=arq[:])
            nc.gpsimd.collective_compute(
                "AllReduce",
                ALU.add,
                replica_groups=[list(range(B))],
                ins=[ar_in[:].opt()],
                outs=[ar_out[:].opt()],
            )
            art = sb.tile([C, 2], F32, name=f"art{round_id}")
            nc.sync.dma_start(out=art[:], in_=ar_out[:])

            inv_m = 1.0 / float(m_count)
            gcol = w[br]["bn"][:, bn_cols[0] : bn_cols[0] + 1]
            bcol = w[br]["bn"][:, bn_cols[1] : bn_cols[1] + 1]
            mean = sb.tile([C, 1], F32, name=f"mean{round_id}_{br}")
            nc.vector.tensor_scalar_mul(mean[:], art[:, 0:1], inv_m)
            ey2 = sb.tile([C, 1], F32, name=f"ey2{round_id}_{br}")
            nc.vector.tensor_scalar_mul(ey2[:], art[:, 1:2], inv_m)
            var = sb.tile([C, 1], F32, name=f"var{round_id}_{br}")
            nc.vector.tensor_tensor(out=var[:], in0=mean[:], in1=mean[:], op=ALU.mult)
            nc.vector.tensor_tensor(out=var[:], in0=ey2[:], in1=var[:], op=ALU.subtract)
            nc.vector.tensor_scalar_add(var[:], var[:], EPS)
            rv = sb.tile([C, 1], F32, name=f"rv{round_id}_{br}")
            nc.vector.reciprocal(rv[:], var[:])
            rstd = sb.tile([C, 1], F32, name=f"rstd{round_id}_{br}")
            nc.scalar.activation(out=rstd[:], in_=rv[:], func=AF.Sqrt)
            a_col = sb.tile([C, 1], F32, name=f"acol{round_id}_{br}")
            nc.vector.tensor_tensor(out=a_col[:], in0=gcol, in1=rstd[:], op=ALU.mult)
            c_col = sb.tile([C, 1], F32, name=f"ccol{round_id}_{br}")
            nc.vector.tensor_tensor(out=c_col[:], in0=mean[:], in1=a_col[:], op=ALU.mult)
            nc.vector.tensor_tensor(out=c_col[:], in0=bcol, in1=c_col[:], op=ALU.subtract)
            return (a_col, c_col)

        # ---------------- conv2 + stats ----------------
        o2_tiles = {}
        stats2 = {}

        def conv2(br, aff):
            a_col, c_col = aff
            ol = []
            s1 = sb.tile([C, NCH], F32, name=f"s1d_{br}")
            s2 = sb.tile([C, NCH], F32, name=f"s2d_{br}")
            for ci, (c0, cn) in enumerate(CHUNKS):
                ot = o1_tiles[br][ci]
                o1r_ap = ot[:, 0:384].rearrange("p (t n) -> p t n", t=3)[:, :, :cn]
                nc.scalar.activation(
                    out=o1r_ap, in_=o1r_ap, func=AF.Relu, scale=a_col[:], bias=c_col[:]
                )
                o2ps = st_ps.tile([C, 128], F32, name=f"o2ps_{br}_{ci}", tag="stage")
                for d in range(3):
                    nc.tensor.matmul(
                        out=o2ps[:, :cn],
                        lhsT=w[br]["w2t"][:, d * C : (d + 1) * C],
                        rhs=ot[:, d * C : d * C + cn],
                        start=(d == 0),
                        stop=(d == 2),
                    )
                o2 = sb.tile([C, C], F32, name=f"o2_{br}_{ci}", tag=f"o2{br}", bufs=NCH)
                nc.scalar.activation(
                    out=o2[:, :cn],
                    in_=o2ps[:, :cn],
                    func=AF.Copy,
                    accum_out=s1[:, ci : ci + 1],
                )
                osq = sb.tile([C, C], F32, name=f"o2sq_{br}_{ci}", tag="o2sq", bufs=2)
                nc.scalar.activation(
                    out=osq[:, :cn],
                    in_=o2ps[:, :cn],
                    func=AF.Square,
                    accum_out=s2[:, ci : ci + 1],
                )
                ol.append(o2)
            o2_tiles[br] = ol
            stats2[br] = (s1, s2)

        # ---------------- emit ----------------
        tables(1)
        knn(x, 1, 1)
        tables(2)
        knn(m, 2, 2)

        tap_adds(1)
        conv1_t(1)
        tap_adds(2)
        conv1_t(2)

        aff1_1 = allreduce_affine(stats1[1], B * N * 3, (0, 1), "1a", 1)
        conv2(1, aff1_1)
        aff1_2 = allreduce_affine(stats1[2], B * N * 3, (0, 1), "1b", 2)
        conv2(2, aff1_2)

        aff2_1 = allreduce_affine(stats2[1], B * N, (2, 3), "2a", 1)
        a1, c1 = aff2_1
        f1_tiles = []
        for ci, (c0, cn) in enumerate(CHUNKS):
            f1t = o2_tiles[1][ci]
            nc.scalar.activation(
                out=f1t[:, :cn],
                in_=f1t[:, :cn],
                func=AF.Relu,
                scale=a1[:],
                bias=c1[:],
            )
            f1_tiles.append(f1t)

        aff2_2 = allreduce_affine(stats2[2], B * N, (2, 3), "2b", 2)
        a2, c2 = aff2_2
        if delta_nonneg:
            a2d = sb.tile([C, 1], F32, name="a2d")
            nc.vector.tensor_tensor(out=a2d[:], in0=a2[:], in1=dcol[:], op=ALU.mult)
            c2d = sb.tile([C, 1], F32, name="c2d")
            nc.vector.tensor_tensor(out=c2d[:], in0=c2[:], in1=dcol[:], op=ALU.mult)
        for ci, (c0, cn) in enumerate(CHUNKS):
            f1t = f1_tiles[ci]
            f2t = sb.tile([C, C], F32, name=f"f2_{ci}", tag="f2", bufs=2)
            if delta_nonneg:
                nc.scalar.activation(
                    out=f2t[:, :cn],
                    in_=o2_tiles[2][ci][:, :cn],
                    func=AF.Relu,
                    scale=a2d[:],
                    bias=c2d[:],
                )
            else:
                nc.scalar.activation(
                    out=f2t[:, :cn],
                    in_=o2_tiles[2][ci][:, :cn],
                    func=AF.Relu,
                    scale=a2[:],
                    bias=c2[:],
                )
                nc.vector.tensor_scalar_mul(f2t[:, :cn], f2t[:, :cn], dcol[:])
            of = sb.tile([C, C], F32, name=f"of_{ci}", tag="of", bufs=2)
            nc.vector.tensor_tensor(
                out=of[:, :cn], in0=f1t[:, :cn], in1=f2t[:, :cn], op=ALU.add
            )
            nc.sync.dma_start(out=out_t[:, c0 : c0 + cn], in_=of[:, :cn])


# ======================= host side =======================

_CACHE = {}


def _prep_branch(w1, b1, g1, be1, w2, b2, g2, be2):
    w1 = np.asarray(w1, dtype=np.float32)
    w2 = np.asarray(w2, dtype=np.float32)
    A = w1[:, :C, 0, :]  # [o, i, 3]
    Bm = w1[:, C:, 0, :]  # [o, i, 3]
    P = (A + Bm).sum(axis=2)  # [o, i]
    blocks = [np.ascontiguousarray((-Bm[:, :, d]).T) for d in range(3)]  # [i, o]
    zA = np.ascontiguousarray((P - Bm[:, :, 0]).T)
    z = np.ascontiguousarray(P.T)
    nbt_ext = np.concatenate(blocks + [zA, z, z], axis=1).astype(np.float16)  # [C,768]
    w2t = np.ascontiguousarray(
        np.concatenate([w2[:, :, 0, d].T for d in range(3)], axis=1)
    ).astype(np.float16)  # [C, 3C]
    bn = np.ascontiguousarray(
        np.stack(
            [
                np.asarray(g1, np.float32),
                np.asarray(be1, np.float32),
                np.asarray(g2, np.float32),
                np.asarray(be2, np.float32),
            ],
            axis=1,
        )
    )  # [C, 4]
    return nbt_ext, w2t, bn


def kernel(**inputs):
    import os

    features = np.ascontiguousarray(np.asarray(inputs["features"], np.float32))
    motion = np.ascontiguousarray(np.asarray(inputs["motion"], np.float32))
    delta = np.asarray(inputs["delta"], np.float32).reshape(-1)[0]

    nbt1, w2t1, bn1 = _prep_branch(
        inputs["d1_w1"], inputs["d1_b1"], inputs["d1_g1"], inputs["d1_be1"],
        inputs["d1_w2"], inputs["d1_b2"], inputs["d1_g2"], inputs["d1_be2"],
    )
    nbt2, w2t2, bn2 = _prep_branch(
        inputs["d2_w1"], inputs["d2_b1"], inputs["d2_g1"], inputs["d2_be1"],
        inputs["d2_w2"], inputs["d2_b2"], inputs["d2_g2"], inputs["d2_be2"],
    )

    delta_nonneg = bool(delta >= 0.0)
    pd_f32r = bool(int(os.environ.get("DG_PD_F32R", "0")))
    key = ("dg", delta_nonneg, pd_f32r)
    if key not in _CACHE:
        _CACHE[key] = build_kernel(delta_nonneg, pd_f32r)
    nc = _CACHE[key]

    shared = {
        "nbt1": nbt1, "w2t1": w2t1, "bn1": bn1,
        "nbt2": nbt2, "w2t2": w2t2, "bn2": bn2,
        "delta": np.array([[delta]], np.float32),
    }
    in_maps = []
    for c in range(B):
        im = dict(shared)
        im["feat"] = np.ascontiguousarray(features[c, :, :, 0])
        im["mot"] = np.ascontiguousarray(motion[c, :, :, 0])
        in_maps.append(im)

    trace = bool(int(os.environ.get("DG_KERNEL_TRACE", "0")))
    res = bass_utils.run_bass_kernel_spmd(
        nc, in_maps, core_ids=list(range(B)), trace=trace
    )
    global LAST_RESULTS
    LAST_RESULTS = res
    out = np.stack([res.results[c]["out"] for c in range(B)], axis=0)
    return out.reshape(B, C, N, 1).astype(np.float32)


LAST_RESULTS = None
